# revision 1
# baseline (speedup 1.0000x reference)
"""MGCN (multi-graph GCN layer) Trainium2 kernel.

Math: with K0/K1/K2 = kernel rows de-interleaved (kernel[d*3+mx, u]),
  out[b] = X[b] @ K0 + bias + A0 @ (X[b] @ K1) + A1 @ (X[b] @ K2)
because the SpMM (over nodes) commutes with the per-feature projection.

Sharding: node-parallel for the SpMM. Core c owns output rows
[c*1250, (c+1)*1250) for ALL 64 batches. Every core redundantly computes the
full projections Y1 = X@K1, Y2 = X@K2 (cheap in bf16 on the PE) and writes
them row-interleaved into a local HBM scratch Y12[2n+s] = Ys[n] of shape
[2N, B*U] bf16, so the SpMM gather needs no cross-core traffic.

Stage 2: per output block of 128 rows, the edges of both supports (grouped by
32-row subgroup, sorted, padded to 128-edge tiles; padded to a uniform tile
count so all 8 cores run one identical SPMD program) are gathered with
dma_gather as full 8KB bf16 rows of Y12 (idx = 2*col + support), and the PE
accumulates segment sums via selector matmuls
  psum_f[32j:32j+32, :] += SelT[128e, 32r].T @ G[128e, f*512:(f+1)*512]
into 8 chunk-PSUM banks (one per group of 8 batches). The X@K0+bias term is
added by small per-(j, batch) matmuls from a per-core xt slice, then each
bank is copied out and written strided into the [B, N, U] output.

The single dma_gather descriptor per edge moves 8KB, which keeps the GpSimd
(SWDGE descriptor generation) cost ~8x below the HBM/DMA time — the kernel is
HBM-bound on the irreducible gather traffic.
"""

import math
from dataclasses import dataclass, field

import numpy as np
import ml_dtypes

import concourse.bass as bass
import concourse.bacc as bacc
import concourse.mybir as mybir
from concourse.tile import TileContext, add_dep_helper

F32 = mybir.dt.float32
BF16 = mybir.dt.bfloat16
FP8 = mybir.dt.float8e4
I16 = mybir.dt.int16


@dataclass
class Cfg:
    B: int = 64          # total batches
    N: int = 10000       # nodes
    D: int = 64          # input features
    U: int = 64          # units
    n_cores: int = 8
    GU: int = 2          # gather-unit size in 128-edge tiles (elem = 8KB);
                         # small units keep 4 col-groups' tiles live at once
    CHUNK: int = 512     # stage-1 node chunk (multiple of 128)
    DMA_SCRATCH: int = 16384
    NQ: int = 4          # SWDGE queues; gathers round-robin across them
    FP8_GATHER: bool = False  # fp8e4m3 gather path: halves DMA but rel err ~3e-2 (too lossy)

    @property
    def GDT(self):       # gather-path dtype
        return FP8 if self.FP8_GATHER else BF16

    @property
    def F(self):         # full feature width B*U
        return self.B * self.U

    @property
    def NPC(self):       # nodes (output rows) per core
        return self.N // self.n_cores

    @property
    def KD(self):        # contraction dim incl. ones row
        return self.D + 1

    @property
    def NT(self):        # stage-1 node tiles of 128 (full projection)
        return (self.N + 127) // 128

    @property
    def NBLK(self):      # per-core output blocks of 128 rows
        return (self.NPC + 127) // 128

    @property
    def NCHUNK(self):    # 512-col feature chunks
        return self.F // 512


@dataclass
class EdgeMeta:
    T: list                         # [blk][j] -> tile count (same all cores)
    idx_off: list                   # [blk][j] -> column offset into idx_all/8
    sel_off: list                   # [blk][j] -> column offset into sel_all/32
    idx_shape: tuple
    sel_shape: tuple


def preprocess_edges(cfg: Cfg, supports):
    """Build per-core idx/sel arrays with a uniform SPMD structure.

    Returns (meta, idx_by_core [n_cores, 128, W_i], sel_by_core).
    Edge (r, c, v) of support s gathers Y12 row 2c+s; it lands in core
    r // NPC, block (r % NPC) // 128, subgroup ((r % NPC) % 128) // 32.
    """
    N, NPC = cfg.N, cfg.NPC
    n_groups_rows = []  # per (core, blk, j): (idx_list, val, lr)
    groups = {}
    for s, (rows, cols, vals) in enumerate(supports):
        rows = np.asarray(rows)
        cols = np.asarray(cols)
        vals = np.asarray(vals, np.float32)
        order = np.argsort(rows, kind="stable")
        r, c, v = rows[order], cols[order], vals[order]
        core = r // NPC
        rr = r % NPC
        blk = rr // 128
        j = (rr % 128) // 32
        lr = rr % 32
        gidx = 2 * c + s
        key = np.stack([core, blk, j])
        for cc in range(cfg.n_cores):
            m0 = core == cc
            for bb in range(cfg.NBLK):
                m1 = m0 & (blk == bb)
                for jj in range(4):
                    m = m1 & (j == jj)
                    if not m.any():
                        continue
                    g = groups.setdefault((cc, bb, jj), [[], [], []])
                    g[0].append(gidx[m])
                    g[1].append(v[m])
                    g[2].append(lr[m])

    # per-(blk, j) tile count: max over cores (keeps SPMD, minimizes padding)
    def glen(key):
        g = groups.get(key)
        return sum(len(a) for a in g[0]) if g else 0

    T = [[0] * 4 for _ in range(cfg.NBLK)]
    for bb in range(cfg.NBLK):
        for jj in range(4):
            mx = max(glen((cc, bb, jj)) for cc in range(cfg.n_cores))
            T[bb][jj] = (mx + 127) // 128

    idx_off = [[0] * 4 for _ in range(cfg.NBLK)]
    sel_off = [[0] * 4 for _ in range(cfg.NBLK)]
    io = so = 0
    for bb in range(cfg.NBLK):
        for jj in range(4):
            idx_off[bb][jj] = io
            sel_off[bb][jj] = so
            io += T[bb][jj] * 8
            so += T[bb][jj] * 32

    idx_by_core, sel_by_core = [], []
    for cc in range(cfg.n_cores):
        idx_cols, sel_cols = [], []
        for bb in range(cfg.NBLK):
            for jj in range(4):
                Tt = T[bb][jj]
                if Tt == 0:
                    continue
                g = groups.get((cc, bb, jj))
                if g is None:
                    gi = np.zeros(0, np.int64)
                    gv = np.zeros(0, np.float32)
                    gl = np.zeros(0, np.int64)
                else:
                    gi = np.concatenate(g[0])
                    gv = np.concatenate(g[1])
                    gl = np.concatenate(g[2])
                pad = Tt * 128 - len(gi)
                gi = np.concatenate([gi, np.zeros(pad, np.int64)])
                gv = np.concatenate([gv, np.zeros(pad, np.float32)])
                gl = np.concatenate([gl, np.zeros(pad, np.int64)])
                # idx wrap: index i -> [i % 16, i // 16], replicated x8
                wrapped = gi.astype(np.int16).reshape(Tt * 8, 16).T
                idx_cols.append(np.tile(wrapped, (8, 1)))
                sel = np.zeros((128, Tt, 32), np.float32)
                lane = np.arange(Tt * 128) % 128
                tt = np.arange(Tt * 128) // 128
                sel[lane, tt, gl] = gv
                gdt = (ml_dtypes.float8_e4m3 if cfg.FP8_GATHER
                       else ml_dtypes.bfloat16)
                sel_cols.append(sel.reshape(128, Tt * 32).astype(gdt))
        idx_by_core.append(np.ascontiguousarray(np.concatenate(idx_cols, axis=1)))
        sel_by_core.append(np.ascontiguousarray(np.concatenate(sel_cols, axis=1)))

    meta = EdgeMeta(T=T, idx_off=idx_off, sel_off=sel_off,
                    idx_shape=idx_by_core[0].shape,
                    sel_shape=sel_by_core[0].shape)
    return meta, idx_by_core, sel_by_core


def prep_weights(cfg: Cfg, kernel, bias):
    K = kernel.reshape(cfg.D, 3, cfg.U)
    kc12 = np.zeros((cfg.KD, 2 * cfg.U), np.float32)
    kc12[:cfg.D, :cfg.U] = K[:, 1]
    kc12[:cfg.D, cfg.U:] = K[:, 2]
    k0b = np.zeros((cfg.KD, cfg.U), np.float32)
    k0b[:cfg.D] = K[:, 0]
    k0b[cfg.D] = bias
    return (kc12.astype(ml_dtypes.bfloat16), k0b.astype(ml_dtypes.bfloat16))


def prep_x(cfg: Cfg, x):
    """x [B, N, D] f32 -> xt_full [KD, B, N] bf16 (d-major, ones row)."""
    xt = np.empty((cfg.KD, cfg.B, cfg.N), np.float32)
    xt[:cfg.D] = x.transpose(2, 0, 1)
    xt[cfg.D] = 1.0
    return np.ascontiguousarray(xt.astype(ml_dtypes.bfloat16))


def prep_x_core(cfg: Cfg, xt_full, core):
    """xt_own [KD, B, NPC] bf16 slice for the X@K0+bias term."""
    sl = xt_full[:, :, core * cfg.NPC:(core + 1) * cfg.NPC]
    return np.ascontiguousarray(sl)


def build_nc(cfg: Cfg, meta: EdgeMeta):
    nc = bacc.Bacc("TRN2", num_devices=cfg.n_cores,
                   dynamic_dma_scratch_size=cfg.DMA_SCRATCH,
                   num_swdge_queues=cfg.NQ)
    KD, F, U, N, B = cfg.KD, cfg.F, cfg.U, cfg.N, cfg.B
    NPC = cfg.NPC

    xt_t = nc.dram_tensor("xt", [KD, B, N], BF16, kind="ExternalInput")
    xo_t = nc.dram_tensor("xo", [KD, B, NPC], BF16, kind="ExternalInput")
    kc12_t = nc.dram_tensor("kc12", [KD, 2 * U], BF16, kind="ExternalInput")
    k0b_t = nc.dram_tensor("k0b", [KD, U], BF16, kind="ExternalInput")
    idx_t = nc.dram_tensor("idx16", list(meta.idx_shape), I16,
                           kind="ExternalInput")
    GDT = cfg.GDT
    sel_t = nc.dram_tensor("sel", list(meta.sel_shape), GDT,
                           kind="ExternalInput")
    y12_t = nc.dram_tensor("y12", [2 * N, F], GDT, kind="Internal")
    out_t = nc.dram_tensor("out", [B, NPC, U], F32, kind="ExternalOutput")

    with TileContext(nc) as tc:
        with tc.tile_pool(name="kpool", bufs=1) as kpool:
            kc_sb = kpool.tile([KD, 2 * U], BF16, tag="kc")
            nc.sync.dma_start(kc_sb[:, :], kc12_t.ap()[:, :])
            k0b_sb = kpool.tile([KD, U], BF16, tag="k0b")
            nc.sync.dma_start(k0b_sb[:, :], k0b_t.ap()[:, :])

            # ---- Stage 1: full projection Y12[2n+s] = (X @ K_{s+1})[n] ----
            y12_writes = []
            with tc.tile_pool(name="xc", bufs=2) as xcpool, \
                 tc.tile_pool(name="st1", bufs=3) as stpool, \
                 tc.tile_pool(name="ps1", bufs=4, space="PSUM") as ps1pool:
                for c0 in range(0, N, cfg.CHUNK):
                    cw = min(cfg.CHUNK, N - c0)
                    xc = xcpool.tile([KD, B, cw], BF16, tag="xc")
                    nc.sync.dma_start(xc[:, :, :], xt_t.ap()[:, :, c0:c0 + cw])
                    for t0 in range(0, cw, 128):
                        nn = min(128, cw - t0)
                        st = stpool.tile([128, 2, F], GDT, tag="st")
                        for b8 in range(B // 8):
                            pp = ps1pool.tile([128, 8, 2 * U], F32, tag="pp")
                            for b2 in range(8):
                                b = b8 * 8 + b2
                                # the tile spans 2 PSUM banks; start clears
                                # one 2KB bank region, so restart per bank
                                nc.tensor.matmul(pp[:nn, b2, :],
                                                 xc[:, b, t0:t0 + nn],
                                                 kc_sb[:, :],
                                                 start=(b2 % 4 == 0),
                                                 stop=(b2 % 4 == 3),
                                                 skip_group_check=True)
                            # pp layout [n, b2, (s u)] -> st [n, s, (b2 u)]
                            nc.any.tensor_copy(
                                st[:nn, :, b8 * 512:b8 * 512 + 512]
                                .rearrange("p s (b2 u) -> p b2 s u", b2=8),
                                pp[:nn, :, :].rearrange(
                                    "p b2 (s u) -> p b2 s u", s=2))
                        n0 = c0 + t0
                        y12v = y12_t.ap().rearrange("(n s) f -> n s f", s=2)
                        y12_writes.append(nc.sync.dma_start(
                            y12v[n0:n0 + nn, 0, :], st[:nn, 0, :]))
                        y12_writes.append(nc.sync.dma_start(
                            y12v[n0:n0 + nn, 1, :], st[:nn, 1, :]))

            # Gate ONLY the gathers on stage 1 (Tile does not track DRAM RAW
            # deps): a nop that depends on every Y12 write, which every
            # gather then depends on. Leaves Y0 matmuls and sel/idx/xtt
            # prefetch free to overlap stage 1.
            y12_done = nc.sync.nop()
            for w in y12_writes:
                add_dep_helper(y12_done.ins, w.ins, sync=True,
                               reason="y12 complete")

            # ---- Stage 2: SpMM + X@K0 + bias, per 128-row block ----
            with tc.tile_pool(name="gp", bufs=6) as gpool, \
                 tc.tile_pool(name="ip", bufs=8) as ipool, \
                 tc.tile_pool(name="sp", bufs=8) as spool, \
                 tc.tile_pool(name="xb", bufs=2) as xbpool, \
                 tc.tile_pool(name="op", bufs=2) as opool, \
                 tc.tile_pool(name="ps2", bufs=1, space="PSUM") as ps2pool:
                gq = 0
                for blk in range(cfg.NBLK):
                    n0 = blk * 128
                    nn = min(128, NPC - n0)
                    groups = [j for j in range(4) if 32 * j < nn]
                    pss = [ps2pool.tile([128, 512], F32, tag=f"ps{f}",
                                        name=f"ps{f}")
                           for f in range(cfg.NCHUNK)]

                    xtt = xbpool.tile([KD, B, 128], BF16, tag="xtt")
                    nc.sync.dma_start(xtt[:, :, :nn],
                                      xo_t.ap()[:, :, n0:n0 + nn])

                    # (out, lhsT, rhs, chunk, j) — interleave across col
                    # groups j so adjacent PE matmuls target different 32-col
                    # strips of the array and execute concurrently.
                    y0_by_j = {j: [] for j in groups}
                    for j in groups:
                        rj = min(32, nn - 32 * j)
                        for b in range(B):
                            y0_by_j[j].append(
                                (pss[b // 8][32 * j:32 * j + rj,
                                             (b % 8) * U:(b % 8 + 1) * U],
                                 xtt[:, b, 32 * j:32 * j + rj],
                                 k0b_sb[:, :], b // 8, j))
                    # issue gathers in the SAME j-interleaved order the
                    # matmuls consume them — pool slots are granted in issue
                    # order, so per-j issue order would deadlock the chain
                    units_by_j = {j: list(range(0, meta.T[blk][j], cfg.GU))
                                  for j in groups}
                    sel_by_j = {j: [] for j in groups}
                    max_units = max((len(u) for u in units_by_j.values()),
                                    default=0)
                    for k in range(max_units):
                        for j in groups:
                            if k >= len(units_by_j[j]):
                                continue
                            u0 = units_by_j[j][k]
                            Tt = meta.T[blk][j]
                            nt = min(cfg.GU, Tt - u0)
                            io = (meta.idx_off[blk][j] + u0 * 8)
                            so = (meta.sel_off[blk][j] + u0 * 32)
                            it = ipool.tile([128, nt * 8], I16, tag="idx")
                            nc.sync.dma_start(it[:, :],
                                              idx_t.ap()[:, io:io + nt * 8])
                            sl = spool.tile([128, nt * 32], GDT, tag="sel")
                            nc.sync.dma_start(sl[:, :],
                                              sel_t.ap()[:, so:so + nt * 32])
                            gt = gpool.tile([128, nt, F], GDT, tag="g")
                            gi_ = nc.gpsimd.dma_gather(
                                gt[:, :, :], y12_t.ap()[:, :], it[:, :],
                                num_idxs=nt * 128, num_idxs_reg=nt * 128,
                                elem_size=F, queue_num=gq % cfg.NQ)
                            add_dep_helper(gi_.ins, y12_done.ins, sync=True,
                                           reason="gather after y12")
                            gq += 1
                            for ti in range(nt):
                                for f in range(cfg.NCHUNK):
                                    sel_by_j[j].append(
                                        (pss[f][32 * j:32 * (j + 1), :],
                                         sl[:, ti * 32:(ti + 1) * 32],
                                         gt[:, ti, f * 512:(f + 1) * 512],
                                         f, j))

                    def interleave(by_j):
                        out = []
                        idxs = {j: 0 for j in by_j}
                        while True:
                            emitted = False
                            for j in by_j:
                                if idxs[j] < len(by_j[j]):
                                    out.append(by_j[j][idxs[j]])
                                    idxs[j] += 1
                                    emitted = True
                            if not emitted:
                                return out

                    specs = interleave(y0_by_j) + interleave(sel_by_j)

                    first = {}
                    last = {}
                    for i, sp in enumerate(specs):
                        first.setdefault((sp[3], sp[4]), i)
                        last[(sp[3], sp[4])] = i
                    prev_mm = None
                    for i, (out_ap, lhsT, rhs, f, j) in enumerate(specs):
                        mm = nc.tensor.matmul(
                            out_ap, lhsT, rhs,
                            start=(first[(f, j)] == i),
                            stop=(last[(f, j)] == i),
                            tile_position=(0, 32 * j),
                            skip_group_check=True)
                        if prev_mm is not None:
                            add_dep_helper(mm.ins, prev_mm.ins, sync=False,
                                           reason="psum accumulation order")
                        prev_mm = mm

                    ot = opool.tile([128, F], F32, tag="ot")
                    for f in range(cfg.NCHUNK):
                        nc.any.tensor_copy(ot[:nn, f * 512:(f + 1) * 512],
                                           pss[f][:nn, :])
                    for b in range(B):
                        nc.scalar.dma_start(out_t.ap()[b, n0:n0 + nn, :],
                                            ot[:nn, b * U:(b + 1) * U])
    return nc


def run(cfg: Cfg, inputs, trace=False, **spmd_kwargs):
    supports = [(np.asarray(inputs["sup0_rows"]), np.asarray(inputs["sup0_cols"]),
                 np.asarray(inputs["sup0_vals"], np.float32)),
                (np.asarray(inputs["sup1_rows"]), np.asarray(inputs["sup1_cols"]),
                 np.asarray(inputs["sup1_vals"], np.float32))]
    meta, idx_by_core, sel_by_core = preprocess_edges(cfg, supports)
    kc12, k0b = prep_weights(cfg, np.asarray(inputs["kernel"], np.float32),
                             np.asarray(inputs["bias"], np.float32))
    xt_full = prep_x(cfg, np.asarray(inputs["x"], np.float32))
    nc = build_nc(cfg, meta)
    nc.compile()
    in_maps = []
    for c in range(cfg.n_cores):
        in_maps.append({
            "xt": xt_full,
            "xo": prep_x_core(cfg, xt_full, c),
            "kc12": kc12,
            "k0b": k0b,
            "idx16": idx_by_core[c],
            "sel": sel_by_core[c],
        })

    from concourse.bass_utils import run_bass_kernel_spmd
    res = run_bass_kernel_spmd(nc, in_maps, core_ids=list(range(cfg.n_cores)),
                               trace=trace, **spmd_kwargs)
    out = np.concatenate([res.results[c]["out"] for c in range(cfg.n_cores)],
                         axis=1)
    return out, res


def kernel(**inputs) -> np.ndarray:
    """Full MGCN layer: takes the unsharded inputs of reference.setup_inputs()
    and returns the full [B, N, UNITS] float32 output."""
    out, _ = run(Cfg(), inputs, trace=False)
    return np.asarray(out, np.float32)



# revision 6
# speedup vs baseline: 1.3576x; 1.3576x over previous
"""MGCN (multi-graph GCN layer) Trainium2 kernel.

Math: with K0/K1/K2 = kernel rows de-interleaved (kernel[d*3+mx, u]),
  out[b] = X[b] @ K0 + bias + A0 @ (X[b] @ K1) + A1 @ (X[b] @ K2)
because the SpMM (over nodes) commutes with the per-feature projection.

Sharding: node-parallel for the SpMM. Core c owns output rows
[c*1250, (c+1)*1250) for ALL 64 batches. Every core redundantly computes the
full projections Y1 = X@K1, Y2 = X@K2 (cheap in bf16 on the PE) and writes
them row-interleaved into a local HBM scratch Y12[2n+s] = Ys[n] of shape
[2N, B*U] bf16, so the SpMM gather needs no cross-core traffic.

Stage 2: per output block of 128 rows, the edges of both supports (grouped by
32-row subgroup, sorted, padded to 128-edge tiles; padded to a uniform tile
count so all 8 cores run one identical SPMD program) are gathered with
dma_gather as full 8KB bf16 rows of Y12 (idx = 2*col + support), and the PE
accumulates segment sums via selector matmuls
  psum_f[32j:32j+32, :] += SelT[128e, 32r].T @ G[128e, f*512:(f+1)*512]
into 8 chunk-PSUM banks (one per group of 8 batches). The X@K0+bias term is
added by small per-(j, batch) matmuls from a per-core xt slice, then each
bank is copied out and written strided into the [B, N, U] output.

The single dma_gather descriptor per edge moves 8KB, which keeps the GpSimd
(SWDGE descriptor generation) cost ~8x below the HBM/DMA time — the kernel is
HBM-bound on the irreducible gather traffic.
"""

import math
from dataclasses import dataclass, field

import numpy as np
import ml_dtypes

import concourse.bass as bass
import concourse.bacc as bacc
import concourse.mybir as mybir
from concourse.tile import TileContext, add_dep_helper

F32 = mybir.dt.float32
BF16 = mybir.dt.bfloat16
FP8E3 = mybir.dt.float8e3
I16 = mybir.dt.int16


@dataclass
class Cfg:
    B: int = 64          # total batches
    N: int = 10000       # nodes
    D: int = 64          # input features
    U: int = 64          # units
    n_cores: int = 8
    GU: int = 2          # gather-unit size in 128-edge tiles (elem = 8KB);
                         # small units keep 4 col-groups' tiles live at once
    CHUNK: int = 512     # stage-1 node chunk (multiple of 128)
    DMA_SCRATCH: int = 16384
    NQ: int = 4          # SWDGE queues; gathers round-robin across them
    # fp8e3m4 gather path: halves y12-write + gather DMA. sel stays bf16
    # (mixed bf16-stationary x fp8e3-moving matmul is exact on the PE),
    # so the only added error is quantizing y12 to e3m4 (~1.8% rms).
    FP8_GATHER: bool = True

    @property
    def GDT(self):       # gather-path dtype
        return FP8E3 if self.FP8_GATHER else BF16

    @property
    def F(self):         # full feature width B*U
        return self.B * self.U

    @property
    def NPC(self):       # nodes (output rows) per core
        return self.N // self.n_cores

    @property
    def KD(self):        # contraction dim incl. ones row
        return self.D + 1

    @property
    def NT(self):        # stage-1 node tiles of 128 (full projection)
        return (self.N + 127) // 128

    @property
    def NBLK(self):      # per-core output blocks of 128 rows
        return (self.NPC + 127) // 128

    @property
    def NCHUNK(self):    # 512-col feature chunks
        return self.F // 512


@dataclass
class EdgeMeta:
    T: list                         # [blk][j] -> tile count (same all cores)
    idx_off: list                   # [blk][j] -> column offset into idx_all/8
    sel_off: list                   # [blk][j] -> column offset into sel_all/32
    idx_shape: tuple
    sel_shape: tuple


def preprocess_edges(cfg: Cfg, supports):
    """Build per-core idx/sel arrays with a uniform SPMD structure.

    Returns (meta, idx_by_core [n_cores, 128, W_i], sel_by_core).
    Edge (r, c, v) of support s gathers Y12 row 2c+s; it lands in core
    r // NPC, block (r % NPC) // 128, subgroup ((r % NPC) % 128) // 32.
    """
    N, NPC = cfg.N, cfg.NPC
    n_groups_rows = []  # per (core, blk, j): (idx_list, val, lr)
    groups = {}
    for s, (rows, cols, vals) in enumerate(supports):
        rows = np.asarray(rows)
        cols = np.asarray(cols)
        vals = np.asarray(vals, np.float32)
        order = np.argsort(rows, kind="stable")
        r, c, v = rows[order], cols[order], vals[order]
        core = r // NPC
        rr = r % NPC
        blk = rr // 128
        j = (rr % 128) // 32
        lr = rr % 32
        gidx = 2 * c + s
        key = np.stack([core, blk, j])
        for cc in range(cfg.n_cores):
            m0 = core == cc
            for bb in range(cfg.NBLK):
                m1 = m0 & (blk == bb)
                for jj in range(4):
                    m = m1 & (j == jj)
                    if not m.any():
                        continue
                    g = groups.setdefault((cc, bb, jj), [[], [], []])
                    g[0].append(gidx[m])
                    g[1].append(v[m])
                    g[2].append(lr[m])

    # per-(blk, j) tile count: max over cores (keeps SPMD, minimizes padding)
    def glen(key):
        g = groups.get(key)
        return sum(len(a) for a in g[0]) if g else 0

    T = [[0] * 4 for _ in range(cfg.NBLK)]
    for bb in range(cfg.NBLK):
        for jj in range(4):
            mx = max(glen((cc, bb, jj)) for cc in range(cfg.n_cores))
            T[bb][jj] = (mx + 127) // 128

    idx_off = [[0] * 4 for _ in range(cfg.NBLK)]
    sel_off = [[0] * 4 for _ in range(cfg.NBLK)]
    io = so = 0
    for bb in range(cfg.NBLK):
        for jj in range(4):
            idx_off[bb][jj] = io
            sel_off[bb][jj] = so
            io += T[bb][jj] * 8
            so += T[bb][jj] * 32

    idx_by_core, sel_by_core = [], []
    for cc in range(cfg.n_cores):
        idx_cols, sel_cols = [], []
        for bb in range(cfg.NBLK):
            for jj in range(4):
                Tt = T[bb][jj]
                if Tt == 0:
                    continue
                g = groups.get((cc, bb, jj))
                if g is None:
                    gi = np.zeros(0, np.int64)
                    gv = np.zeros(0, np.float32)
                    gl = np.zeros(0, np.int64)
                else:
                    gi = np.concatenate(g[0])
                    gv = np.concatenate(g[1])
                    gl = np.concatenate(g[2])
                pad = Tt * 128 - len(gi)
                gi = np.concatenate([gi, np.zeros(pad, np.int64)])
                gv = np.concatenate([gv, np.zeros(pad, np.float32)])
                gl = np.concatenate([gl, np.zeros(pad, np.int64)])
                # idx wrap: index i -> [i % 16, i // 16], replicated x8
                wrapped = gi.astype(np.int16).reshape(Tt * 8, 16).T
                idx_cols.append(np.tile(wrapped, (8, 1)))
                sel = np.zeros((128, Tt, 32), np.float32)
                lane = np.arange(Tt * 128) % 128
                tt = np.arange(Tt * 128) // 128
                sel[lane, tt, gl] = gv
                # sel stays bf16 even on the fp8 gather path (mixed matmul)
                sel_cols.append(sel.reshape(128, Tt * 32)
                                .astype(ml_dtypes.bfloat16))
        idx_by_core.append(np.ascontiguousarray(np.concatenate(idx_cols, axis=1)))
        sel_by_core.append(np.ascontiguousarray(np.concatenate(sel_cols, axis=1)))

    meta = EdgeMeta(T=T, idx_off=idx_off, sel_off=sel_off,
                    idx_shape=idx_by_core[0].shape,
                    sel_shape=sel_by_core[0].shape)
    return meta, idx_by_core, sel_by_core


def prep_weights(cfg: Cfg, kernel, bias):
    K = kernel.reshape(cfg.D, 3, cfg.U)
    kc12 = np.zeros((cfg.KD, 2 * cfg.U), np.float32)
    kc12[:cfg.D, :cfg.U] = K[:, 1]
    kc12[:cfg.D, cfg.U:] = K[:, 2]
    k0b = np.zeros((cfg.KD, cfg.U), np.float32)
    k0b[:cfg.D] = K[:, 0]
    k0b[cfg.D] = bias
    return (kc12.astype(ml_dtypes.bfloat16), k0b.astype(ml_dtypes.bfloat16))


def prep_x(cfg: Cfg, x):
    """x [B, N, D] f32 -> xt_full [KD, B, N] bf16 (d-major, ones row)."""
    xt = np.empty((cfg.KD, cfg.B, cfg.N), np.float32)
    xt[:cfg.D] = x.transpose(2, 0, 1)
    xt[cfg.D] = 1.0
    return np.ascontiguousarray(xt.astype(ml_dtypes.bfloat16))


def prep_x_core(cfg: Cfg, xt_full, core):
    """xt_own [KD, B, NPC] bf16 slice for the X@K0+bias term."""
    sl = xt_full[:, :, core * cfg.NPC:(core + 1) * cfg.NPC]
    return np.ascontiguousarray(sl)


def build_nc(cfg: Cfg, meta: EdgeMeta):
    nc = bacc.Bacc("TRN2", num_devices=cfg.n_cores,
                   dynamic_dma_scratch_size=cfg.DMA_SCRATCH,
                   num_swdge_queues=cfg.NQ)
    KD, F, U, N, B = cfg.KD, cfg.F, cfg.U, cfg.N, cfg.B
    NPC = cfg.NPC

    xt_t = nc.dram_tensor("xt", [KD, B, N], BF16, kind="ExternalInput")
    xo_t = nc.dram_tensor("xo", [KD, B, NPC], BF16, kind="ExternalInput")
    kc12_t = nc.dram_tensor("kc12", [KD, 2 * U], BF16, kind="ExternalInput")
    k0b_t = nc.dram_tensor("k0b", [KD, U], BF16, kind="ExternalInput")
    idx_t = nc.dram_tensor("idx16", list(meta.idx_shape), I16,
                           kind="ExternalInput")
    GDT = cfg.GDT
    sel_t = nc.dram_tensor("sel", list(meta.sel_shape), BF16,
                           kind="ExternalInput")
    y12_t = nc.dram_tensor("y12", [2 * N, F], GDT, kind="Internal")
    out_t = nc.dram_tensor("out", [B, NPC, U], F32, kind="ExternalOutput")

    with TileContext(nc) as tc:
        with tc.tile_pool(name="kpool", bufs=1) as kpool:
            kc_sb = kpool.tile([KD, 2 * U], BF16, tag="kc")
            nc.sync.dma_start(kc_sb[:, :], kc12_t.ap()[:, :])
            k0b_sb = kpool.tile([KD, U], BF16, tag="k0b")
            nc.sync.dma_start(k0b_sb[:, :], k0b_t.ap()[:, :])

            # ---- Stage 1: full projection Y12[2n+s] = (X @ K_{s+1})[n] ----
            y12_writes = []
            with tc.tile_pool(name="xc", bufs=2) as xcpool, \
                 tc.tile_pool(name="st1", bufs=3) as stpool, \
                 tc.tile_pool(name="ps1", bufs=4, space="PSUM") as ps1pool:
                for c0 in range(0, N, cfg.CHUNK):
                    cw = min(cfg.CHUNK, N - c0)
                    xc = xcpool.tile([KD, B, cw], BF16, tag="xc")
                    nc.sync.dma_start(xc[:, :, :], xt_t.ap()[:, :, c0:c0 + cw])
                    for t0 in range(0, cw, 128):
                        nn = min(128, cw - t0)
                        st = stpool.tile([128, 2, F], GDT, tag="st")
                        for b8 in range(B // 8):
                            pp = ps1pool.tile([128, 8, 2 * U], F32, tag="pp")
                            for b2 in range(8):
                                b = b8 * 8 + b2
                                # the tile spans 2 PSUM banks; start clears
                                # one 2KB bank region, so restart per bank
                                nc.tensor.matmul(pp[:nn, b2, :],
                                                 xc[:, b, t0:t0 + nn],
                                                 kc_sb[:, :],
                                                 start=(b2 % 4 == 0),
                                                 stop=(b2 % 4 == 3),
                                                 skip_group_check=True)
                            # pp layout [n, b2, (s u)] -> st [n, s, (b2 u)]
                            nc.any.tensor_copy(
                                st[:nn, :, b8 * 512:b8 * 512 + 512]
                                .rearrange("p s (b2 u) -> p b2 s u", b2=8),
                                pp[:nn, :, :].rearrange(
                                    "p b2 (s u) -> p b2 s u", s=2))
                        n0 = c0 + t0
                        y12v = y12_t.ap().rearrange("(n s) f -> n s f", s=2)
                        y12_writes.append(nc.sync.dma_start(
                            y12v[n0:n0 + nn, 0, :], st[:nn, 0, :]))
                        y12_writes.append(nc.sync.dma_start(
                            y12v[n0:n0 + nn, 1, :], st[:nn, 1, :]))

            # Gate ONLY the gathers on stage 1 (Tile does not track DRAM RAW
            # deps): a nop that depends on every Y12 write, which every
            # gather then depends on. Leaves Y0 matmuls and sel/idx/xtt
            # prefetch free to overlap stage 1.
            y12_done = nc.sync.nop()
            for w in y12_writes:
                add_dep_helper(y12_done.ins, w.ins, sync=True,
                               reason="y12 complete")

            # ---- Stage 2: SpMM + X@K0 + bias, per 128-row block ----
            with tc.tile_pool(name="gp", bufs=6) as gpool, \
                 tc.tile_pool(name="ip", bufs=8) as ipool, \
                 tc.tile_pool(name="sp", bufs=8) as spool, \
                 tc.tile_pool(name="xb", bufs=2) as xbpool, \
                 tc.tile_pool(name="op", bufs=2) as opool, \
                 tc.tile_pool(name="ps2", bufs=1, space="PSUM") as ps2pool:
                gq = 0
                for blk in range(cfg.NBLK):
                    n0 = blk * 128
                    nn = min(128, NPC - n0)
                    groups = [j for j in range(4) if 32 * j < nn]
                    pss = [ps2pool.tile([128, 512], F32, tag=f"ps{f}",
                                        name=f"ps{f}")
                           for f in range(cfg.NCHUNK)]

                    xtt = xbpool.tile([KD, B, 128], BF16, tag="xtt")
                    nc.sync.dma_start(xtt[:, :, :nn],
                                      xo_t.ap()[:, :, n0:n0 + nn])

                    # (out, lhsT, rhs, chunk, j) — interleave across col
                    # groups j so adjacent PE matmuls target different 32-col
                    # strips of the array and execute concurrently.
                    y0_by_j = {j: [] for j in groups}
                    for j in groups:
                        rj = min(32, nn - 32 * j)
                        for b in range(B):
                            y0_by_j[j].append(
                                (pss[b // 8][32 * j:32 * j + rj,
                                             (b % 8) * U:(b % 8 + 1) * U],
                                 xtt[:, b, 32 * j:32 * j + rj],
                                 k0b_sb[:, :], b // 8, j))
                    # issue gathers in the SAME j-interleaved order the
                    # matmuls consume them — pool slots are granted in issue
                    # order, so per-j issue order would deadlock the chain
                    units_by_j = {j: list(range(0, meta.T[blk][j], cfg.GU))
                                  for j in groups}
                    sel_by_j = {j: [] for j in groups}
                    max_units = max((len(u) for u in units_by_j.values()),
                                    default=0)
                    for k in range(max_units):
                        for j in groups:
                            if k >= len(units_by_j[j]):
                                continue
                            u0 = units_by_j[j][k]
                            Tt = meta.T[blk][j]
                            nt = min(cfg.GU, Tt - u0)
                            io = (meta.idx_off[blk][j] + u0 * 8)
                            so = (meta.sel_off[blk][j] + u0 * 32)
                            it = ipool.tile([128, nt * 8], I16, tag="idx")
                            nc.sync.dma_start(it[:, :],
                                              idx_t.ap()[:, io:io + nt * 8])
                            sl = spool.tile([128, nt * 32], BF16, tag="sel")
                            nc.sync.dma_start(sl[:, :],
                                              sel_t.ap()[:, so:so + nt * 32])
                            gt = gpool.tile([128, nt, F], GDT, tag="g")
                            gi_ = nc.gpsimd.dma_gather(
                                gt[:, :, :], y12_t.ap()[:, :], it[:, :],
                                num_idxs=nt * 128, num_idxs_reg=nt * 128,
                                elem_size=F, queue_num=gq % cfg.NQ)
                            add_dep_helper(gi_.ins, y12_done.ins, sync=True,
                                           reason="gather after y12")
                            gq += 1
                            for ti in range(nt):
                                for f in range(cfg.NCHUNK):
                                    sel_by_j[j].append(
                                        (pss[f][32 * j:32 * (j + 1), :],
                                         sl[:, ti * 32:(ti + 1) * 32],
                                         gt[:, ti, f * 512:(f + 1) * 512],
                                         f, j))

                    def interleave(by_j):
                        out = []
                        idxs = {j: 0 for j in by_j}
                        while True:
                            emitted = False
                            for j in by_j:
                                if idxs[j] < len(by_j[j]):
                                    out.append(by_j[j][idxs[j]])
                                    idxs[j] += 1
                                    emitted = True
                            if not emitted:
                                return out

                    specs = interleave(y0_by_j) + interleave(sel_by_j)

                    first = {}
                    last = {}
                    for i, sp in enumerate(specs):
                        first.setdefault((sp[3], sp[4]), i)
                        last[(sp[3], sp[4])] = i
                    prev_mm = None
                    for i, (out_ap, lhsT, rhs, f, j) in enumerate(specs):
                        mm = nc.tensor.matmul(
                            out_ap, lhsT, rhs,
                            start=(first[(f, j)] == i),
                            stop=(last[(f, j)] == i),
                            tile_position=(0, 32 * j),
                            skip_group_check=True)
                        if prev_mm is not None:
                            add_dep_helper(mm.ins, prev_mm.ins, sync=False,
                                           reason="psum accumulation order")
                        prev_mm = mm

                    ot = opool.tile([128, F], F32, tag="ot")
                    for f in range(cfg.NCHUNK):
                        nc.any.tensor_copy(ot[:nn, f * 512:(f + 1) * 512],
                                           pss[f][:nn, :])
                    for b in range(B):
                        nc.scalar.dma_start(out_t.ap()[b, n0:n0 + nn, :],
                                            ot[:nn, b * U:(b + 1) * U])
    return nc


def run(cfg: Cfg, inputs, trace=False, **spmd_kwargs):
    supports = [(np.asarray(inputs["sup0_rows"]), np.asarray(inputs["sup0_cols"]),
                 np.asarray(inputs["sup0_vals"], np.float32)),
                (np.asarray(inputs["sup1_rows"]), np.asarray(inputs["sup1_cols"]),
                 np.asarray(inputs["sup1_vals"], np.float32))]
    meta, idx_by_core, sel_by_core = preprocess_edges(cfg, supports)
    kc12, k0b = prep_weights(cfg, np.asarray(inputs["kernel"], np.float32),
                             np.asarray(inputs["bias"], np.float32))
    xt_full = prep_x(cfg, np.asarray(inputs["x"], np.float32))
    nc = build_nc(cfg, meta)
    nc.compile()
    in_maps = []
    for c in range(cfg.n_cores):
        in_maps.append({
            "xt": xt_full,
            "xo": prep_x_core(cfg, xt_full, c),
            "kc12": kc12,
            "k0b": k0b,
            "idx16": idx_by_core[c],
            "sel": sel_by_core[c],
        })

    from concourse.bass_utils import run_bass_kernel_spmd
    res = run_bass_kernel_spmd(nc, in_maps, core_ids=list(range(cfg.n_cores)),
                               trace=trace, **spmd_kwargs)
    out = np.concatenate([res.results[c]["out"] for c in range(cfg.n_cores)],
                         axis=1)
    return out, res


def kernel(**inputs) -> np.ndarray:
    """Full MGCN layer: takes the unsharded inputs of reference.setup_inputs()
    and returns the full [B, N, UNITS] float32 output."""
    out, _ = run(Cfg(), inputs, trace=False)
    return np.asarray(out, np.float32)



# revision 9
# speedup vs baseline: 2.0586x; 1.5164x over previous
"""MGCN (multi-graph GCN layer) Trainium2 kernel.

Math: with K0/K1/K2 = kernel rows de-interleaved (kernel[d*3+mx, u]),
  out[b] = X[b] @ K0 + bias + A0 @ (X[b] @ K1) + A1 @ (X[b] @ K2)
because the SpMM (over nodes) commutes with the per-feature projection.

Sharding: node-parallel for the SpMM. Core c owns output rows
[c*1250, (c+1)*1250) for ALL 64 batches. Every core redundantly computes the
full projections Y1 = X@K1, Y2 = X@K2 (cheap in bf16 on the PE) and writes
them row-interleaved into a local HBM scratch Y12[2n+s] = Ys[n] of shape
[2N, B*U] bf16, so the SpMM gather needs no cross-core traffic.

Stage 2: per output block of 128 rows, the edges of both supports (grouped by
32-row subgroup, sorted, padded to 128-edge tiles; padded to a uniform tile
count so all 8 cores run one identical SPMD program) are gathered with
dma_gather as full 8KB bf16 rows of Y12 (idx = 2*col + support), and the PE
accumulates segment sums via selector matmuls
  psum_f[32j:32j+32, :] += SelT[128e, 32r].T @ G[128e, f*512:(f+1)*512]
into 8 chunk-PSUM banks (one per group of 8 batches). The X@K0+bias term is
added by small per-(j, batch) matmuls from a per-core xt slice, then each
bank is copied out and written strided into the [B, N, U] output.

The single dma_gather descriptor per edge moves 8KB, which keeps the GpSimd
(SWDGE descriptor generation) cost ~8x below the HBM/DMA time — the kernel is
HBM-bound on the irreducible gather traffic.
"""

import math
from dataclasses import dataclass, field

import numpy as np
import ml_dtypes

import concourse.bass as bass
import concourse.bacc as bacc
import concourse.mybir as mybir
from concourse.tile import TileContext, add_dep_helper

F32 = mybir.dt.float32
BF16 = mybir.dt.bfloat16
FP8E3 = mybir.dt.float8e3
I16 = mybir.dt.int16


@dataclass
class Cfg:
    B: int = 64          # total batches
    N: int = 10000       # nodes
    D: int = 64          # input features
    U: int = 64          # units
    n_cores: int = 8
    GU: int = 2          # gather-unit size in 128-edge tiles (elem = 8KB);
                         # small units keep 4 col-groups' tiles live at once
    CHUNK: int = 512     # stage-1 node chunk (multiple of 128)
    DMA_SCRATCH: int = 16384
    NQ: int = 4          # SWDGE queues; gathers round-robin across them
    # fp8e3m4 gather path: halves y12-write + gather DMA. sel stays bf16
    # (mixed bf16-stationary x fp8e3-moving matmul is exact on the PE),
    # so the only added error is quantizing y12 to e3m4 (~1.8% rms).
    FP8_GATHER: bool = True
    # Scale y12 by YSCALE before the e3m4 cast (and sel by 1/YSCALE) to push
    # y12 (sigma~0.4) out of e3m4's denormal range (tiny=0.25).
    YSCALE: float = 4.0

    @property
    def GDT(self):       # gather-path dtype
        return FP8E3 if self.FP8_GATHER else BF16

    @property
    def F(self):         # full feature width B*U
        return self.B * self.U

    @property
    def NPC(self):       # nodes (output rows) per core
        return self.N // self.n_cores

    @property
    def KD(self):        # contraction dim incl. ones row
        return self.D + 1

    @property
    def NT(self):        # stage-1 node tiles of 128 (full projection)
        return (self.N + 127) // 128

    @property
    def NBLK(self):      # per-core output blocks of 128 rows
        return (self.NPC + 127) // 128

    @property
    def NCHUNK(self):    # 512-col feature chunks
        return self.F // 512


@dataclass
class EdgeMeta:
    T: list                         # [blk][j] -> tile count (same all cores)
    idx_off: list                   # [blk][j] -> column offset into idx_all/8
    sel_off: list                   # [blk][j] -> column offset into sel_all/32
    idx_shape: tuple
    sel_shape: tuple


def preprocess_edges(cfg: Cfg, supports):
    """Build per-core idx/sel arrays with a uniform SPMD structure.

    Returns (meta, idx_by_core [n_cores, 128, W_i], sel_by_core).
    Edge (r, c, v) of support s gathers Y12 row 2c+s; it lands in core
    r // NPC, block (r % NPC) // 128, subgroup ((r % NPC) % 128) // 32.
    """
    N, NPC = cfg.N, cfg.NPC
    n_groups_rows = []  # per (core, blk, j): (idx_list, val, lr)
    groups = {}
    for s, (rows, cols, vals) in enumerate(supports):
        rows = np.asarray(rows)
        cols = np.asarray(cols)
        vals = np.asarray(vals, np.float32)
        order = np.argsort(rows, kind="stable")
        r, c, v = rows[order], cols[order], vals[order]
        core = r // NPC
        rr = r % NPC
        blk = rr // 128
        j = (rr % 128) // 32
        lr = rr % 32
        gidx = 2 * c + s
        key = np.stack([core, blk, j])
        for cc in range(cfg.n_cores):
            m0 = core == cc
            for bb in range(cfg.NBLK):
                m1 = m0 & (blk == bb)
                for jj in range(4):
                    m = m1 & (j == jj)
                    if not m.any():
                        continue
                    g = groups.setdefault((cc, bb, jj), [[], [], []])
                    g[0].append(gidx[m])
                    g[1].append(v[m])
                    g[2].append(lr[m])

    # per-(blk, j) tile count: max over cores (keeps SPMD, minimizes padding)
    def glen(key):
        g = groups.get(key)
        return sum(len(a) for a in g[0]) if g else 0

    T = [[0] * 4 for _ in range(cfg.NBLK)]
    for bb in range(cfg.NBLK):
        for jj in range(4):
            mx = max(glen((cc, bb, jj)) for cc in range(cfg.n_cores))
            T[bb][jj] = (mx + 127) // 128

    idx_off = [[0] * 4 for _ in range(cfg.NBLK)]
    sel_off = [[0] * 4 for _ in range(cfg.NBLK)]
    io = so = 0
    for bb in range(cfg.NBLK):
        for jj in range(4):
            idx_off[bb][jj] = io
            sel_off[bb][jj] = so
            io += T[bb][jj] * 8
            so += T[bb][jj] * 32

    idx_by_core, sel_by_core = [], []
    for cc in range(cfg.n_cores):
        idx_cols, sel_cols = [], []
        for bb in range(cfg.NBLK):
            for jj in range(4):
                Tt = T[bb][jj]
                if Tt == 0:
                    continue
                g = groups.get((cc, bb, jj))
                if g is None:
                    gi = np.zeros(0, np.int64)
                    gv = np.zeros(0, np.float32)
                    gl = np.zeros(0, np.int64)
                else:
                    gi = np.concatenate(g[0])
                    gv = np.concatenate(g[1])
                    gl = np.concatenate(g[2])
                pad = Tt * 128 - len(gi)
                gi = np.concatenate([gi, np.zeros(pad, np.int64)])
                gv = np.concatenate([gv, np.zeros(pad, np.float32)])
                gl = np.concatenate([gl, np.zeros(pad, np.int64)])
                # idx wrap: index i -> [i % 16, i // 16], replicated x8
                wrapped = gi.astype(np.int16).reshape(Tt * 8, 16).T
                idx_cols.append(np.tile(wrapped, (8, 1)))
                sel = np.zeros((128, Tt, 32), np.float32)
                lane = np.arange(Tt * 128) % 128
                tt = np.arange(Tt * 128) // 128
                sel[lane, tt, gl] = gv
                # sel stays bf16 even on the fp8 gather path (mixed matmul)
                sel_cols.append(sel.reshape(128, Tt * 32)
                                .astype(ml_dtypes.bfloat16))
        idx_by_core.append(np.ascontiguousarray(np.concatenate(idx_cols, axis=1)))
        sel_by_core.append(np.ascontiguousarray(np.concatenate(sel_cols, axis=1)))

    meta = EdgeMeta(T=T, idx_off=idx_off, sel_off=sel_off,
                    idx_shape=idx_by_core[0].shape,
                    sel_shape=sel_by_core[0].shape)
    return meta, idx_by_core, sel_by_core


def prep_weights(cfg: Cfg, kernel, bias):
    K = kernel.reshape(cfg.D, 3, cfg.U)
    ys = cfg.YSCALE if cfg.FP8_GATHER else 1.0
    kc12 = np.zeros((cfg.KD, 2 * cfg.U), np.float32)
    kc12[:cfg.D, :cfg.U] = K[:, 1] * ys
    kc12[:cfg.D, cfg.U:] = K[:, 2] * ys
    k0b = np.zeros((cfg.KD, cfg.U), np.float32)
    k0b[:cfg.D] = K[:, 0]
    k0b[cfg.D] = bias
    return (kc12.astype(ml_dtypes.bfloat16), k0b.astype(ml_dtypes.bfloat16))


def prep_x(cfg: Cfg, x):
    """x [B, N, D] f32 -> xt_full [KD, B, N] bf16 (d-major, ones row)."""
    xt = np.empty((cfg.KD, cfg.B, cfg.N), np.float32)
    xt[:cfg.D] = x.transpose(2, 0, 1)
    xt[cfg.D] = 1.0
    return np.ascontiguousarray(xt.astype(ml_dtypes.bfloat16))


def prep_x_core(cfg: Cfg, xt_full, core):
    """xt_own [KD, B, NPC] bf16 slice for the X@K0+bias term."""
    sl = xt_full[:, :, core * cfg.NPC:(core + 1) * cfg.NPC]
    return np.ascontiguousarray(sl)


def build_nc(cfg: Cfg, meta: EdgeMeta):
    nc = bacc.Bacc("TRN2", num_devices=cfg.n_cores,
                   dynamic_dma_scratch_size=cfg.DMA_SCRATCH,
                   num_swdge_queues=cfg.NQ)
    KD, F, U, N, B = cfg.KD, cfg.F, cfg.U, cfg.N, cfg.B
    NPC = cfg.NPC

    xt_t = nc.dram_tensor("xt", [KD, B, N], BF16, kind="ExternalInput")
    xo_t = nc.dram_tensor("xo", [KD, B, NPC], BF16, kind="ExternalInput")
    kc12_t = nc.dram_tensor("kc12", [KD, 2 * U], BF16, kind="ExternalInput")
    k0b_t = nc.dram_tensor("k0b", [KD, U], BF16, kind="ExternalInput")
    idx_t = nc.dram_tensor("idx16", list(meta.idx_shape), I16,
                           kind="ExternalInput")
    GDT = cfg.GDT
    sel_t = nc.dram_tensor("sel", list(meta.sel_shape), BF16,
                           kind="ExternalInput")
    y12_t = nc.dram_tensor("y12", [2 * N, F], GDT, kind="Internal")
    out_t = nc.dram_tensor("out", [B, NPC, U], F32, kind="ExternalOutput")

    with TileContext(nc) as tc:
        with tc.tile_pool(name="kpool", bufs=1) as kpool:
            kc_sb = kpool.tile([KD, 2 * U], BF16, tag="kc")
            nc.sync.dma_start(kc_sb[:, :], kc12_t.ap()[:, :])
            k0b_sb = kpool.tile([KD, U], BF16, tag="k0b")
            nc.sync.dma_start(k0b_sb[:, :], k0b_t.ap()[:, :])

            # ---- Stage 1: full projection Y12[2n+s] = (X @ K_{s+1})[n] ----
            y12_writes = []
            with tc.tile_pool(name="xc", bufs=2) as xcpool, \
                 tc.tile_pool(name="st1", bufs=3) as stpool, \
                 tc.tile_pool(name="ps1", bufs=4, space="PSUM") as ps1pool:
                for c0 in range(0, N, cfg.CHUNK):
                    cw = min(cfg.CHUNK, N - c0)
                    xc = xcpool.tile([KD, B, cw], BF16, tag="xc")
                    nc.sync.dma_start(xc[:, :, :], xt_t.ap()[:, :, c0:c0 + cw])
                    for t0 in range(0, cw, 128):
                        nn = min(128, cw - t0)
                        st = stpool.tile([128, 2, F], GDT, tag="st")
                        for b8 in range(B // 8):
                            pp = ps1pool.tile([128, 8, 2 * U], F32, tag="pp")
                            for b2 in range(8):
                                b = b8 * 8 + b2
                                # the tile spans 2 PSUM banks; start clears
                                # one 2KB bank region, so restart per bank
                                nc.tensor.matmul(pp[:nn, b2, :],
                                                 xc[:, b, t0:t0 + nn],
                                                 kc_sb[:, :],
                                                 start=(b2 % 4 == 0),
                                                 stop=(b2 % 4 == 3),
                                                 skip_group_check=True)
                            # pp layout [n, b2, (s u)] -> st [n, s, (b2 u)]
                            nc.any.tensor_copy(
                                st[:nn, :, b8 * 512:b8 * 512 + 512]
                                .rearrange("p s (b2 u) -> p b2 s u", b2=8),
                                pp[:nn, :, :].rearrange(
                                    "p b2 (s u) -> p b2 s u", s=2))
                        n0 = c0 + t0
                        y12v = y12_t.ap().rearrange("(n s) f -> n s f", s=2)
                        y12_writes.append(nc.sync.dma_start(
                            y12v[n0:n0 + nn, 0, :], st[:nn, 0, :]))
                        y12_writes.append(nc.sync.dma_start(
                            y12v[n0:n0 + nn, 1, :], st[:nn, 1, :]))

            # Gate ONLY the gathers on stage 1 (Tile does not track DRAM RAW
            # deps): a nop that depends on every Y12 write, which every
            # gather then depends on. Leaves Y0 matmuls and sel/idx/xtt
            # prefetch free to overlap stage 1.
            y12_done = nc.sync.nop()
            for w in y12_writes:
                add_dep_helper(y12_done.ins, w.ins, sync=True,
                               reason="y12 complete")

            # ---- Stage 2: SpMM + X@K0 + bias, per 128-row block ----
            with tc.tile_pool(name="gp", bufs=6) as gpool, \
                 tc.tile_pool(name="ip", bufs=8) as ipool, \
                 tc.tile_pool(name="sp", bufs=8) as spool, \
                 tc.tile_pool(name="xb", bufs=2) as xbpool, \
                 tc.tile_pool(name="op", bufs=2) as opool, \
                 tc.tile_pool(name="ps2", bufs=1, space="PSUM") as ps2pool:
                gq = 0
                for blk in range(cfg.NBLK):
                    n0 = blk * 128
                    nn = min(128, NPC - n0)
                    groups = [j for j in range(4) if 32 * j < nn]
                    pss = [ps2pool.tile([128, 512], F32, tag=f"ps{f}",
                                        name=f"ps{f}")
                           for f in range(cfg.NCHUNK)]

                    xtt = xbpool.tile([KD, B, 128], BF16, tag="xtt")
                    nc.sync.dma_start(xtt[:, :, :nn],
                                      xo_t.ap()[:, :, n0:n0 + nn])

                    # (out, lhsT, rhs, chunk, j) — interleave across col
                    # groups j so adjacent PE matmuls target different 32-col
                    # strips of the array and execute concurrently.
                    y0_by_j = {j: [] for j in groups}
                    for j in groups:
                        rj = min(32, nn - 32 * j)
                        for b in range(B):
                            y0_by_j[j].append(
                                (pss[b // 8][32 * j:32 * j + rj,
                                             (b % 8) * U:(b % 8 + 1) * U],
                                 xtt[:, b, 32 * j:32 * j + rj],
                                 k0b_sb[:, :], b // 8, j))
                    # issue gathers in the SAME j-interleaved order the
                    # matmuls consume them — pool slots are granted in issue
                    # order, so per-j issue order would deadlock the chain
                    units_by_j = {j: list(range(0, meta.T[blk][j], cfg.GU))
                                  for j in groups}
                    sel_by_j = {j: [] for j in groups}
                    max_units = max((len(u) for u in units_by_j.values()),
                                    default=0)
                    for k in range(max_units):
                        for j in groups:
                            if k >= len(units_by_j[j]):
                                continue
                            u0 = units_by_j[j][k]
                            Tt = meta.T[blk][j]
                            nt = min(cfg.GU, Tt - u0)
                            io = (meta.idx_off[blk][j] + u0 * 8)
                            so = (meta.sel_off[blk][j] + u0 * 32)
                            it = ipool.tile([128, nt * 8], I16, tag="idx")
                            nc.sync.dma_start(it[:, :],
                                              idx_t.ap()[:, io:io + nt * 8])
                            sl = spool.tile([128, nt * 32], BF16, tag="sel")
                            nc.sync.dma_start(sl[:, :],
                                              sel_t.ap()[:, so:so + nt * 32])
                            gt = gpool.tile([128, nt, F], GDT, tag="g")
                            gi_ = nc.gpsimd.dma_gather(
                                gt[:, :, :], y12_t.ap()[:, :], it[:, :],
                                num_idxs=nt * 128, num_idxs_reg=nt * 128,
                                elem_size=F, queue_num=gq % cfg.NQ)
                            add_dep_helper(gi_.ins, y12_done.ins, sync=True,
                                           reason="gather after y12")
                            gq += 1
                            for ti in range(nt):
                                for f in range(cfg.NCHUNK):
                                    sel_by_j[j].append(
                                        (pss[f][32 * j:32 * (j + 1), :],
                                         sl[:, ti * 32:(ti + 1) * 32],
                                         gt[:, ti, f * 512:(f + 1) * 512],
                                         f, j))

                    def interleave(by_j):
                        out = []
                        idxs = {j: 0 for j in by_j}
                        while True:
                            emitted = False
                            for j in by_j:
                                if idxs[j] < len(by_j[j]):
                                    out.append(by_j[j][idxs[j]])
                                    idxs[j] += 1
                                    emitted = True
                            if not emitted:
                                return out

                    specs = interleave(y0_by_j) + interleave(sel_by_j)

                    first = {}
                    last = {}
                    for i, sp in enumerate(specs):
                        first.setdefault((sp[3], sp[4]), i)
                        last[(sp[3], sp[4])] = i
                    prev_mm = None
                    for i, (out_ap, lhsT, rhs, f, j) in enumerate(specs):
                        mm = nc.tensor.matmul(
                            out_ap, lhsT, rhs,
                            start=(first[(f, j)] == i),
                            stop=(last[(f, j)] == i),
                            tile_position=(0, 32 * j),
                            skip_group_check=True)
                        if prev_mm is not None:
                            add_dep_helper(mm.ins, prev_mm.ins, sync=False,
                                           reason="psum accumulation order")
                        prev_mm = mm

                    ot = opool.tile([128, F], F32, tag="ot")
                    for f in range(cfg.NCHUNK):
                        nc.any.tensor_copy(ot[:nn, f * 512:(f + 1) * 512],
                                           pss[f][:nn, :])
                    for b in range(B):
                        nc.scalar.dma_start(out_t.ap()[b, n0:n0 + nn, :],
                                            ot[:nn, b * U:(b + 1) * U])
    return nc


def run(cfg: Cfg, inputs, trace=False, **spmd_kwargs):
    vs = 1.0 / cfg.YSCALE if cfg.FP8_GATHER else 1.0
    supports = [(np.asarray(inputs["sup0_rows"]), np.asarray(inputs["sup0_cols"]),
                 np.asarray(inputs["sup0_vals"], np.float32) * vs),
                (np.asarray(inputs["sup1_rows"]), np.asarray(inputs["sup1_cols"]),
                 np.asarray(inputs["sup1_vals"], np.float32) * vs)]
    meta, idx_by_core, sel_by_core = preprocess_edges(cfg, supports)
    kc12, k0b = prep_weights(cfg, np.asarray(inputs["kernel"], np.float32),
                             np.asarray(inputs["bias"], np.float32))
    xt_full = prep_x(cfg, np.asarray(inputs["x"], np.float32))
    nc = build_nc(cfg, meta)
    nc.compile()
    in_maps = []
    for c in range(cfg.n_cores):
        in_maps.append({
            "xt": xt_full,
            "xo": prep_x_core(cfg, xt_full, c),
            "kc12": kc12,
            "k0b": k0b,
            "idx16": idx_by_core[c],
            "sel": sel_by_core[c],
        })

    from concourse.bass_utils import run_bass_kernel_spmd
    res = run_bass_kernel_spmd(nc, in_maps, core_ids=list(range(cfg.n_cores)),
                               trace=trace, **spmd_kwargs)
    out = np.concatenate([res.results[c]["out"] for c in range(cfg.n_cores)],
                         axis=1)
    return out, res


def kernel(**inputs) -> np.ndarray:
    """Full MGCN layer: takes the unsharded inputs of reference.setup_inputs()
    and returns the full [B, N, UNITS] float32 output."""
    out, _ = run(Cfg(), inputs, trace=False)
    return np.asarray(out, np.float32)



# revision 10
# speedup vs baseline: 2.2636x; 1.0996x over previous
"""MGCN Trainium2 kernel v2: direct-X gather in fp8e3, no stage 1.

Math: out[b] = X[b] @ K0 + bias + A0 @ X[b] @ K1 + A1 @ X[b] @ K2.
The SpMM commutes with the projection, so instead of gathering rows of the
pre-projected Y12 table (which requires an on-device stage 1 writing a
replicated 164MB table), each core gathers rows of X0 [N, B*D] directly,
quantized to fp8e3m4 on the host (4KB/row instead of 8KB bf16). The sel
matrices stay bf16 — the PE computes mixed bf16-stationary x fp8e3-moving
matmuls exactly, so the only quantization error is e3m4 on X (~1.8% rms on
the SpMM terms; measured end-to-end ~1e-2 rel).

Sharding: node-parallel. Core c owns output rows [c*1250, (c+1)*1250).
Rows are processed in half-blocks of 64: Z1 (support 0) accumulates in PSUM
partitions 0:64 and Z2 in 64:128 of the same 8 chunk banks [128, 512] f32,
one strip q = 2*s + (r%64)//32 per 32 PE columns (tile_position), exactly
like the baseline's 4-way column-strip concurrency.

Per half-block: gathers (idx = source col, elem 4KB) + sel matmuls accumulate
Z1/Z2; the psum is copied to SBUF bf16, transposed per 2-batch group on the
PE (into recycled psum banks), and projected out[r, u] = X^T-slice @ K0b +
Z1^T @ K1 + Z2^T @ K2 per batch into recycled psum chunks; the [nn, B*U]
result is written with one contiguous DMA per half-block into out [NPC, B*U].
"""

import numpy as np
import ml_dtypes

import concourse.bass as bass
import concourse.bacc as bacc
import concourse.mybir as mybir
from concourse.tile import TileContext, add_dep_helper

F32 = mybir.dt.float32
BF16 = mybir.dt.bfloat16
FP8E3 = mybir.dt.float8e3
I16 = mybir.dt.int16

B, N, D, U = 64, 10000, 64, 64
NCORES = 8
NPC = N // NCORES            # 1250 rows per core
HB = 64                      # half-block rows
NBH = (NPC + HB - 1) // HB   # 20 half-blocks (last has 34 rows)
F = B * D                    # 4096 gather-row features
KD = D + 1                   # contraction incl. ones row
NCHUNK = F // 512            # 8 psum column chunks
GU = 4                       # gather unit: tiles of 128 edges per dma_gather
NQ = 4


class Meta:
    pass


def preprocess_edges(supports):
    """Bucket edges by (core, hb, q) with q = 2*support + (row%64)//32,
    pad to a uniform per-(hb,q) tile count across cores (SPMD).

    Returns meta, idx_by_core [NCORES][128, IW] i16, sel_by_core [128, SW] bf16.
    """
    groups = {}
    for s, (rows, cols, vals) in enumerate(supports):
        rows = np.asarray(rows)
        cols = np.asarray(cols)
        vals = np.asarray(vals, np.float32)
        order = np.argsort(rows, kind="stable")
        r, c, v = rows[order], cols[order], vals[order]
        core = r // NPC
        rr = r % NPC
        hb = rr // HB
        q = 2 * s + (rr % HB) // 32
        lr = rr % 32
        for cc in range(NCORES):
            m0 = core == cc
            for bb in range(NBH):
                m1 = m0 & (hb == bb)
                for jj in (2 * s, 2 * s + 1):
                    m = m1 & (q == jj)
                    if not m.any():
                        continue
                    g = groups.setdefault((cc, bb, jj), [[], [], []])
                    g[0].append(c[m])
                    g[1].append(v[m])
                    g[2].append(lr[m])

    def glen(key):
        g = groups.get(key)
        return sum(len(a) for a in g[0]) if g else 0

    T = [[0] * 4 for _ in range(NBH)]
    for bb in range(NBH):
        for jj in range(4):
            mx = max(glen((cc, bb, jj)) for cc in range(NCORES))
            T[bb][jj] = (mx + 127) // 128

    # column offsets into the per-core idx/sel arrays, ordered (hb, q, tile)
    idx_off = [[0] * 4 for _ in range(NBH)]
    sel_off = [[0] * 4 for _ in range(NBH)]
    io = so = 0
    for bb in range(NBH):
        for jj in range(4):
            idx_off[bb][jj] = io
            sel_off[bb][jj] = so
            io += T[bb][jj] * 8
            so += T[bb][jj] * 32

    idx_by_core, sel_by_core = [], []
    for cc in range(NCORES):
        idx_cols, sel_cols = [], []
        for bb in range(NBH):
            for jj in range(4):
                Tt = T[bb][jj]
                if Tt == 0:
                    continue
                g = groups.get((cc, bb, jj))
                if g is None:
                    gi = np.zeros(0, np.int64)
                    gv = np.zeros(0, np.float32)
                    gl = np.zeros(0, np.int64)
                else:
                    gi = np.concatenate(g[0])
                    gv = np.concatenate(g[1])
                    gl = np.concatenate(g[2])
                pad = Tt * 128 - len(gi)
                gi = np.concatenate([gi, np.zeros(pad, np.int64)])
                gv = np.concatenate([gv, np.zeros(pad, np.float32)])
                gl = np.concatenate([gl, np.zeros(pad, np.int64)])
                # idx wrap: index i -> [i % 16, i // 16], replicated x8
                wrapped = gi.astype(np.int16).reshape(Tt * 8, 16).T
                idx_cols.append(np.tile(wrapped, (8, 1)))
                sel = np.zeros((128, Tt, 32), np.float32)
                lane = np.arange(Tt * 128) % 128
                tt = np.arange(Tt * 128) // 128
                sel[lane, tt, gl] = gv
                sel_cols.append(sel.reshape(128, Tt * 32)
                                .astype(ml_dtypes.bfloat16))
        idx_by_core.append(np.ascontiguousarray(np.concatenate(idx_cols, axis=1)))
        sel_by_core.append(np.ascontiguousarray(np.concatenate(sel_cols, axis=1)))

    meta = Meta()
    meta.T = T
    meta.idx_off = idx_off
    meta.sel_off = sel_off
    meta.idx_shape = idx_by_core[0].shape
    meta.sel_shape = sel_by_core[0].shape
    # per-hb idx window (all 4 q groups contiguous) for one DMA per hb
    meta.hb_idx_off = [idx_off[bb][0] for bb in range(NBH)]
    meta.hb_idx_w = [sum(T[bb][jj] * 8 for jj in range(4)) for bb in range(NBH)]
    meta.TQmax = max(max(row) for row in T)
    meta.IWmax = max(meta.hb_idx_w)
    return meta, idx_by_core, sel_by_core


def prep_inputs(inputs):
    x = np.asarray(inputs["x"], np.float32)
    kernel = np.asarray(inputs["kernel"], np.float32)
    bias = np.asarray(inputs["bias"], np.float32)

    # x0q [N, B*D] fp8e3: x0q[n, b*D+d] = x[b, n, d]
    x0 = np.ascontiguousarray(x.transpose(1, 0, 2).reshape(N, B * D))
    x0q = x0.astype(ml_dtypes.float8_e3m4)

    # xo [KD, B, N] bf16 (d-major with ones row) for the identity/proj lhsT
    xt = np.empty((KD, B, N), np.float32)
    xt[:D] = x.transpose(2, 0, 1)
    xt[D] = 1.0
    xt = xt.astype(ml_dtypes.bfloat16)

    K = kernel.reshape(D, 3, U)
    k0b = np.zeros((KD, U), np.float32)
    k0b[:D] = K[:, 0]
    k0b[D] = bias
    k1 = np.ascontiguousarray(K[:, 1])
    k2 = np.ascontiguousarray(K[:, 2])
    ident = np.eye(128, dtype=np.float32)
    return (x0q, xt,
            k0b.astype(ml_dtypes.bfloat16), k1.astype(ml_dtypes.bfloat16),
            k2.astype(ml_dtypes.bfloat16), ident.astype(ml_dtypes.bfloat16))


def build_nc(meta):
    nc = bacc.Bacc("TRN2", num_devices=NCORES,
                   dynamic_dma_scratch_size=16384,
                   num_swdge_queues=NQ)

    x0q_t = nc.dram_tensor("x0q", [N, F], FP8E3, kind="ExternalInput")
    xo_t = nc.dram_tensor("xo", [KD, B, NPC], BF16, kind="ExternalInput")
    k0b_t = nc.dram_tensor("k0b", [KD, U], BF16, kind="ExternalInput")
    k1_t = nc.dram_tensor("k1", [D, U], BF16, kind="ExternalInput")
    k2_t = nc.dram_tensor("k2", [D, U], BF16, kind="ExternalInput")
    id_t = nc.dram_tensor("ident", [128, 128], BF16, kind="ExternalInput")
    idx_t = nc.dram_tensor("idx16", list(meta.idx_shape), I16,
                           kind="ExternalInput")
    sel_t = nc.dram_tensor("sel", list(meta.sel_shape), BF16,
                           kind="ExternalInput")
    out_t = nc.dram_tensor("out", [NPC, B * U], F32, kind="ExternalOutput")

    with TileContext(nc) as tc:
        with tc.tile_pool(name="kpool", bufs=1) as kpool, \
             tc.tile_pool(name="gp", bufs=7) as gpool, \
             tc.tile_pool(name="ip", bufs=2) as ipool, \
             tc.tile_pool(name="sp", bufs=8) as spool, \
             tc.tile_pool(name="xb", bufs=2) as xbpool, \
             tc.tile_pool(name="zb", bufs=2) as zbpool, \
             tc.tile_pool(name="zt", bufs=3) as ztpool, \
             tc.tile_pool(name="op", bufs=2) as opool, \
             tc.tile_pool(name="ps", bufs=1, space="PSUM") as pspool:

            k0b_sb = kpool.tile([KD, U], BF16, tag="k0b")
            nc.sync.dma_start(k0b_sb[:, :], k0b_t.ap()[:, :])
            k1_sb = kpool.tile([D, U], BF16, tag="k1")
            nc.sync.dma_start(k1_sb[:, :], k1_t.ap()[:, :])
            k2_sb = kpool.tile([D, U], BF16, tag="k2")
            nc.sync.dma_start(k2_sb[:, :], k2_t.ap()[:, :])
            id_sb = kpool.tile([128, 128], BF16, tag="id")
            nc.sync.dma_start(id_sb[:, :], id_t.ap()[:, :])

            prev_mm = [None]

            def mm(*args, **kwargs):
                m = nc.tensor.matmul(*args, skip_group_check=True, **kwargs)
                if prev_mm[0] is not None:
                    add_dep_helper(m.ins, prev_mm[0].ins, sync=False,
                                   reason="pe order")
                prev_mm[0] = m
                return m

            gq = 0
            for hb in range(NBH):
                n0 = hb * HB
                nn = min(HB, NPC - n0)
                Ths = meta.T[hb]
                qs = [q for q in range(4) if Ths[q] > 0]

                # psum Z chunks: partitions 0:64 = Z1 rows, 64:128 = Z2 rows
                pss = [pspool.tile([128, 512], F32, tag=f"ps{f}",
                                   name=f"ps{f}_{hb}")
                       for f in range(NCHUNK)]

                xtt = xbpool.tile([KD, B, HB], BF16, tag="xtt")
                nc.sync.dma_start(xtt[:, :, :nn],
                                  xo_t.ap()[:, :, n0:n0 + nn])

                # idx for the whole hb in one DMA; sel per (hb, q)
                iw = meta.hb_idx_w[hb]
                it = ipool.tile([128, meta.IWmax], I16, tag="idx")
                nc.sync.dma_start(it[:, :iw],
                                  idx_t.ap()[:, meta.hb_idx_off[hb]:
                                             meta.hb_idx_off[hb] + iw])
                sls = {}
                for q in qs:
                    sl = spool.tile([128, meta.TQmax * 32], BF16, tag="sel")
                    so = meta.sel_off[hb][q]
                    nc.sync.dma_start(sl[:, :Ths[q] * 32],
                                      sel_t.ap()[:, so:so + Ths[q] * 32])
                    sls[q] = sl

                # gathers + sel matmuls, interleaved across q strips
                mm_specs = {q: [] for q in qs}
                units = {q: list(range(0, Ths[q], GU)) for q in qs}
                for k in range(max(len(u) for u in units.values())):
                    for q in qs:
                        if k >= len(units[q]):
                            continue
                        u0 = units[q][k]
                        nt = min(GU, Ths[q] - u0)
                        io = meta.idx_off[hb][q] - meta.hb_idx_off[hb] + u0 * 8
                        gt = gpool.tile([128, GU, F], FP8E3, tag="g")
                        nc.gpsimd.dma_gather(
                            gt[:, :nt, :], x0q_t.ap()[:, :],
                            it[:, io:io + nt * 8],
                            num_idxs=nt * 128, num_idxs_reg=nt * 128,
                            elem_size=F, queue_num=gq % NQ)
                        gq += 1
                        for ti in range(nt):
                            for f in range(NCHUNK):
                                mm_specs[q].append(
                                    (sls[q][:, (u0 + ti) * 32:
                                            (u0 + ti + 1) * 32],
                                     gt[:, ti, f * 512:(f + 1) * 512], f))

                # emit interleaved across q, chained for psum ordering
                idxs = {q: 0 for q in qs}
                cnt = {}
                total = {q: len(mm_specs[q]) for q in qs}
                remaining = sum(total.values())
                while remaining:
                    for q in qs:
                        i = idxs[q]
                        if i >= total[q]:
                            continue
                        sel_ap, g_ap, f = mm_specs[q][i]
                        c = cnt.get((q, f), 0)
                        nmm = total[q] // NCHUNK
                        mm(pss[f][32 * q:32 * (q + 1), :], sel_ap, g_ap,
                           start=(c == 0), stop=(c == nmm - 1),
                           tile_position=(0, 32 * q))
                        cnt[(q, f)] = c + 1
                        idxs[q] += 1
                        remaining -= 1

                # Z psum -> SBUF bf16 (releases ps banks chunk by chunk)
                zsb = zbpool.tile([128, F], BF16, tag="zsb")
                for f in range(NCHUNK):
                    nc.any.tensor_copy(zsb[:, f * 512:(f + 1) * 512],
                                       pss[f][:, :])

                # transposes + projection per 8-batch group g.
                # transpose zsb[:, b*64:(b+1)*64] [128=(s,r), 64=d] ->
                # zt [64=d, 128=(s,r)] per single batch, so every projection
                # operand sits at partition offset 0.
                ot = opool.tile([HB, F], F32, tag="ot")
                for g in range(8):
                    ztp = pspool.tile([128, 512], F32, tag=f"ps{g % 2}",
                                      name=f"zt{g % 2}_{hb}_{g}")
                    ztv = ztp[:, :].bitcast(BF16)          # [128, 1024]
                    for bloc in range(8):
                        b = 8 * g + bloc
                        mm(ztv[0:D, 128 * bloc:128 * (bloc + 1)],
                           zsb[:, b * D:(b + 1) * D], id_sb[:, :],
                           is_transpose=True)
                    zts = ztpool.tile([D, 1024], BF16, tag="zt")
                    nc.any.tensor_copy(zts[:, :], ztv[0:D, :])

                    ops = pspool.tile([128, 512], F32, tag=f"ps{(g + 2) % 8}",
                                      name=f"out{g}_{hb}")
                    for bloc in range(8):
                        b = 8 * g + bloc
                        oap = ops[:nn, bloc * U:(bloc + 1) * U]
                        zbase = 128 * bloc
                        mm(oap, xtt[:, b, :nn], k0b_sb[:, :], start=True,
                           stop=False)
                        mm(oap, zts[:, zbase:zbase + nn],
                           k1_sb[:, :], start=False, stop=False)
                        mm(oap, zts[:, zbase + D:zbase + D + nn],
                           k2_sb[:, :], start=False, stop=True)
                    nc.any.tensor_copy(ot[:nn, g * 512:(g + 1) * 512],
                                       ops[:nn, :])

                nc.sync.dma_start(out_t.ap()[n0:n0 + nn, :], ot[:nn, :])
    return nc


def run(inputs, trace=False, **spmd_kwargs):
    supports = [(np.asarray(inputs["sup0_rows"]), np.asarray(inputs["sup0_cols"]),
                 np.asarray(inputs["sup0_vals"], np.float32)),
                (np.asarray(inputs["sup1_rows"]), np.asarray(inputs["sup1_cols"]),
                 np.asarray(inputs["sup1_vals"], np.float32))]
    meta, idx_by_core, sel_by_core = preprocess_edges(supports)
    x0q, xt, k0b, k1, k2, ident = prep_inputs(inputs)

    nc = build_nc(meta)
    nc.compile()
    in_maps = []
    for c in range(NCORES):
        in_maps.append({
            "x0q": x0q,
            "xo": np.ascontiguousarray(xt[:, :, c * NPC:(c + 1) * NPC]),
            "k0b": k0b,
            "k1": k1,
            "k2": k2,
            "ident": ident,
            "idx16": idx_by_core[c],
            "sel": sel_by_core[c],
        })

    from concourse.bass_utils import run_bass_kernel_spmd
    res = run_bass_kernel_spmd(nc, in_maps, core_ids=list(range(NCORES)),
                               trace=trace, **spmd_kwargs)
    # out [NPC, B*U] per core -> [B, N, U]
    out = np.concatenate([np.asarray(res.results[c]["out"])
                          .reshape(NPC, B, U) for c in range(NCORES)], axis=0)
    out = np.ascontiguousarray(out.transpose(1, 0, 2))
    return out, res


def kernel(**inputs) -> np.ndarray:
    out, _ = run(inputs, trace=False)
    return np.asarray(out, np.float32)


# revision 11
# speedup vs baseline: 2.8436x; 1.2562x over previous
"""MGCN Trainium2 kernel v3: direct-X fp8e3 gather, 128-row blocks,
support-split SpMM passes.

Math: out[b] = X[b] @ K0 + bias + A0 @ X[b] @ K1 + A1 @ X[b] @ K2.
The SpMM commutes with the projection, so each core gathers rows of
X0 [N, B*D] quantized to fp8e3m4 on the host (4KB/row). sel matrices stay
bf16 — mixed bf16-stationary x fp8e3-moving matmuls are exact on the PE, so
the only quantization error is e3m4 on X (~1.3% rms; ~1.3e-2 rel measured).

Sharding: node-parallel, core c owns rows [c*1250, (c+1)*1250), processed in
10 blocks of 128 rows. Per block, TWO SpMM passes (support 0 then support 1)
accumulate Z_s [128, 4096] f32 into the same 8 psum banks, 4 row-strips of 32
via tile_position. After each pass the psum drains to SBUF bf16; then 2-batch
PE transposes produce Zt [(parity,d), r] views packed 8-per-bank, and a
per-batch projection (3 matmuls: X-part K=65, Z1 K=64, Z2 K=64) writes
out chunks, stored [NPC, B*U] so each block's result is one contiguous DMA.
"""

import numpy as np
import ml_dtypes

import concourse.bass as bass
import concourse.bacc as bacc
import concourse.mybir as mybir
from concourse.tile import TileContext, add_dep_helper

F32 = mybir.dt.float32
BF16 = mybir.dt.bfloat16
FP8E3 = mybir.dt.float8e3
I16 = mybir.dt.int16

B, N, D, U = 64, 10000, 64, 64
NCORES = 8
NPC = N // NCORES            # 1250 rows per core
BLK = 128                    # block rows
NB = (NPC + BLK - 1) // BLK  # 10 blocks (last has 98 rows)
F = B * D                    # 4096 gather-row features
KD = D + 1                   # contraction incl. ones row
NCHUNK = F // 512            # 8 psum column chunks
GU = 4                       # gather unit: tiles of 128 edges per dma_gather
NQ = 4


class Meta:
    pass


def preprocess_edges(supports):
    """Bucket edges by (core, blk, s, j) with j = (row%128)//32, pad to a
    uniform per-(blk,s,j) tile count across cores (SPMD)."""
    groups = {}
    for s, (rows, cols, vals) in enumerate(supports):
        rows = np.asarray(rows)
        cols = np.asarray(cols)
        vals = np.asarray(vals, np.float32)
        order = np.argsort(rows, kind="stable")
        r, c, v = rows[order], cols[order], vals[order]
        core = r // NPC
        rr = r % NPC
        blk = rr // BLK
        j = (rr % BLK) // 32
        lr = rr % 32
        for cc in range(NCORES):
            m0 = core == cc
            for bb in range(NB):
                m1 = m0 & (blk == bb)
                for jj in range(4):
                    m = m1 & (j == jj)
                    if not m.any():
                        continue
                    g = groups.setdefault((cc, bb, s, jj), [[], [], []])
                    g[0].append(c[m])
                    g[1].append(v[m])
                    g[2].append(lr[m])

    def glen(key):
        g = groups.get(key)
        return sum(len(a) for a in g[0]) if g else 0

    # T[blk][s][j]
    T = [[[0] * 4 for _ in range(2)] for _ in range(NB)]
    for bb in range(NB):
        for s in range(2):
            for jj in range(4):
                mx = max(glen((cc, bb, s, jj)) for cc in range(NCORES))
                T[bb][s][jj] = (mx + 127) // 128

    idx_off = [[[0] * 4 for _ in range(2)] for _ in range(NB)]
    sel_off = [[[0] * 4 for _ in range(2)] for _ in range(NB)]
    io = so = 0
    for bb in range(NB):
        for s in range(2):
            for jj in range(4):
                idx_off[bb][s][jj] = io
                sel_off[bb][s][jj] = so
                io += T[bb][s][jj] * 8
                so += T[bb][s][jj] * 32

    idx_by_core, sel_by_core = [], []
    for cc in range(NCORES):
        idx_cols, sel_cols = [], []
        for bb in range(NB):
            for s in range(2):
                for jj in range(4):
                    Tt = T[bb][s][jj]
                    if Tt == 0:
                        continue
                    g = groups.get((cc, bb, s, jj))
                    if g is None:
                        gi = np.zeros(0, np.int64)
                        gv = np.zeros(0, np.float32)
                        gl = np.zeros(0, np.int64)
                    else:
                        gi = np.concatenate(g[0])
                        gv = np.concatenate(g[1])
                        gl = np.concatenate(g[2])
                    pad = Tt * 128 - len(gi)
                    gi = np.concatenate([gi, np.zeros(pad, np.int64)])
                    gv = np.concatenate([gv, np.zeros(pad, np.float32)])
                    gl = np.concatenate([gl, np.zeros(pad, np.int64)])
                    # idx wrap: index i -> [i % 16, i // 16], replicated x8
                    wrapped = gi.astype(np.int16).reshape(Tt * 8, 16).T
                    idx_cols.append(np.tile(wrapped, (8, 1)))
                    sel = np.zeros((128, Tt, 32), np.float32)
                    lane = np.arange(Tt * 128) % 128
                    tt = np.arange(Tt * 128) // 128
                    sel[lane, tt, gl] = gv
                    sel_cols.append(sel.reshape(128, Tt * 32)
                                    .astype(ml_dtypes.bfloat16))
        idx_by_core.append(np.ascontiguousarray(
            np.concatenate(idx_cols, axis=1)))
        sel_by_core.append(np.ascontiguousarray(
            np.concatenate(sel_cols, axis=1)))

    meta = Meta()
    meta.T = T
    meta.idx_off = idx_off
    meta.sel_off = sel_off
    meta.idx_shape = idx_by_core[0].shape
    meta.sel_shape = sel_by_core[0].shape
    meta.blk_idx_off = [idx_off[bb][0][0] for bb in range(NB)]
    meta.blk_idx_w = [sum(T[bb][s][jj] * 8 for s in range(2)
                          for jj in range(4)) for bb in range(NB)]
    meta.TQmax = max(T[bb][s][jj] for bb in range(NB) for s in range(2)
                     for jj in range(4))
    meta.IWmax = max(meta.blk_idx_w)
    return meta, idx_by_core, sel_by_core


def prep_inputs(inputs):
    x = np.asarray(inputs["x"], np.float32)
    kernel = np.asarray(inputs["kernel"], np.float32)
    bias = np.asarray(inputs["bias"], np.float32)

    x0 = np.ascontiguousarray(x.transpose(1, 0, 2).reshape(N, B * D))
    x0q = x0.astype(ml_dtypes.float8_e3m4)

    xt = np.empty((KD, B, N), np.float32)
    xt[:D] = x.transpose(2, 0, 1)
    xt[D] = 1.0
    xt = xt.astype(ml_dtypes.bfloat16)

    K = kernel.reshape(D, 3, U)
    k0b = np.zeros((KD, U), np.float32)
    k0b[:D] = K[:, 0]
    k0b[D] = bias
    # duplicated along partitions so rhs base_partition can match the
    # lhsT parity offset (0 or 64) in the projection matmuls
    k1 = np.ascontiguousarray(np.vstack([K[:, 1], K[:, 1]]))
    k2 = np.ascontiguousarray(np.vstack([K[:, 2], K[:, 2]]))
    ident = np.eye(128, dtype=np.float32)
    return (x0q, xt,
            k0b.astype(ml_dtypes.bfloat16), k1.astype(ml_dtypes.bfloat16),
            k2.astype(ml_dtypes.bfloat16), ident.astype(ml_dtypes.bfloat16))


def build_nc(meta):
    nc = bacc.Bacc("TRN2", num_devices=NCORES,
                   dynamic_dma_scratch_size=16384,
                   num_swdge_queues=NQ)

    x0q_t = nc.dram_tensor("x0q", [N, F], FP8E3, kind="ExternalInput")
    xo_t = nc.dram_tensor("xo", [KD, B, NPC], BF16, kind="ExternalInput")
    k0b_t = nc.dram_tensor("k0b", [KD, U], BF16, kind="ExternalInput")
    k1_t = nc.dram_tensor("k1", [2 * D, U], BF16, kind="ExternalInput")
    k2_t = nc.dram_tensor("k2", [2 * D, U], BF16, kind="ExternalInput")
    id_t = nc.dram_tensor("ident", [128, 128], BF16, kind="ExternalInput")
    idx_t = nc.dram_tensor("idx16", list(meta.idx_shape), I16,
                           kind="ExternalInput")
    sel_t = nc.dram_tensor("sel", list(meta.sel_shape), BF16,
                           kind="ExternalInput")
    out_t = nc.dram_tensor("out", [NPC, B * U], F32, kind="ExternalOutput")

    with TileContext(nc) as tc:
        with tc.tile_pool(name="kpool", bufs=1) as kpool, \
             tc.tile_pool(name="gp", bufs=6) as gpool, \
             tc.tile_pool(name="ip", bufs=2) as ipool, \
             tc.tile_pool(name="sp", bufs=10) as spool, \
             tc.tile_pool(name="xb", bufs=2) as xbpool, \
             tc.tile_pool(name="zb", bufs=1) as zbpool, \
             tc.tile_pool(name="zt", bufs=8) as ztpool, \
             tc.tile_pool(name="op", bufs=2) as opool, \
             tc.tile_pool(name="ps", bufs=1, space="PSUM") as pspool:

            k0b_sb = kpool.tile([KD, U], BF16, tag="k0b")
            nc.sync.dma_start(k0b_sb[:, :], k0b_t.ap()[:, :])
            k1_sb = kpool.tile([2 * D, U], BF16, tag="k1")
            nc.sync.dma_start(k1_sb[:, :], k1_t.ap()[:, :])
            k2_sb = kpool.tile([2 * D, U], BF16, tag="k2")
            nc.sync.dma_start(k2_sb[:, :], k2_t.ap()[:, :])
            id_sb = kpool.tile([128, 128], BF16, tag="id")
            nc.sync.dma_start(id_sb[:, :], id_t.ap()[:, :])

            prev_mm = [None]

            def mm(*args, **kwargs):
                m = nc.tensor.matmul(*args, skip_group_check=True, **kwargs)
                if prev_mm[0] is not None:
                    add_dep_helper(m.ins, prev_mm[0].ins, sync=False,
                                   reason="pe order")
                prev_mm[0] = m
                return m

            gq = 0
            for blk in range(NB):
                n0 = blk * BLK
                nn = min(BLK, NPC - n0)

                xtt = xbpool.tile([KD, B, BLK], BF16, tag="xtt")
                nc.sync.dma_start(xtt[:, :, :nn],
                                  xo_t.ap()[:, :, n0:n0 + nn])

                iw = meta.blk_idx_w[blk]
                it = ipool.tile([128, meta.IWmax], I16, tag="idx")
                nc.sync.dma_start(it[:, :iw],
                                  idx_t.ap()[:, meta.blk_idx_off[blk]:
                                             meta.blk_idx_off[blk] + iw])

                zsbs = {}
                for s in range(2):
                    Ths = meta.T[blk][s]
                    qs = [q for q in range(4) if Ths[q] > 0]
                    pss = [pspool.tile([128, 512], F32, tag=f"ps{f}",
                                       name=f"z{s}c{f}_{blk}")
                           for f in range(NCHUNK)]

                    sls = {}
                    for q in qs:
                        sl = spool.tile([128, meta.TQmax * 32], BF16,
                                        tag="sel")
                        so = meta.sel_off[blk][s][q]
                        nc.sync.dma_start(sl[:, :Ths[q] * 32],
                                          sel_t.ap()[:, so:so + Ths[q] * 32])
                        sls[q] = sl

                    mm_specs = {q: [] for q in qs}
                    units = {q: list(range(0, Ths[q], GU)) for q in qs}
                    for k in range(max(len(u) for u in units.values())):
                        for q in qs:
                            if k >= len(units[q]):
                                continue
                            u0 = units[q][k]
                            nt = min(GU, Ths[q] - u0)
                            io = (meta.idx_off[blk][s][q]
                                  - meta.blk_idx_off[blk] + u0 * 8)
                            gt = gpool.tile([128, GU, F], FP8E3, tag="g")
                            nc.gpsimd.dma_gather(
                                gt[:, :nt, :], x0q_t.ap()[:, :],
                                it[:, io:io + nt * 8],
                                num_idxs=nt * 128, num_idxs_reg=nt * 128,
                                elem_size=F, queue_num=gq % NQ)
                            gq += 1
                            for ti in range(nt):
                                for f in range(NCHUNK):
                                    mm_specs[q].append(
                                        (sls[q][:, (u0 + ti) * 32:
                                                (u0 + ti + 1) * 32],
                                         gt[:, ti, f * 512:(f + 1) * 512], f))

                    idxs = {q: 0 for q in qs}
                    cnt = {}
                    total = {q: len(mm_specs[q]) for q in qs}
                    remaining = sum(total.values())
                    while remaining:
                        for q in qs:
                            i = idxs[q]
                            if i >= total[q]:
                                continue
                            sel_ap, g_ap, f = mm_specs[q][i]
                            c = cnt.get((q, f), 0)
                            nmm = total[q] // NCHUNK
                            mm(pss[f][32 * q:32 * (q + 1), :], sel_ap, g_ap,
                               start=(c == 0), stop=(c == nmm - 1),
                               tile_position=(0, 32 * q))
                            cnt[(q, f)] = c + 1
                            idxs[q] += 1
                            remaining -= 1

                    zsb = zbpool.tile([128, F], BF16, tag=f"zsb{s}")
                    for f in range(NCHUNK):
                        nc.any.tensor_copy(zsb[:, f * 512:(f + 1) * 512],
                                           pss[f][:, :])
                    zsbs[s] = zsb

                # 2-batch transposes: zt psum view [128, 1024] bf16 packs 8
                # transposes = 16 batches; tags: s=0 -> ps0..3, s=1 -> ps4..7
                zts = {}
                for s in range(2):
                    for h in range(4):
                        ztp = pspool.tile([128, 512], F32,
                                          tag=f"ps{4 * s + h}",
                                          name=f"zt{s}_{h}_{blk}")
                        ztv = ztp[:, :].bitcast(BF16)
                        for k in range(8):
                            b2 = 16 * h + 2 * k
                            mm(ztv[:, 128 * k:128 * (k + 1)],
                               zsbs[s][:, b2 * D:(b2 + 2) * D], id_sb[:, :],
                               is_transpose=True)
                        zs = ztpool.tile([128, 1024], BF16, tag="zt")
                        nc.any.tensor_copy(zs[:, :], ztv[:, :])
                        zts[(s, h)] = zs

                # projection: out chunk c serves batches 8c..8c+7
                ot = opool.tile([BLK, F], F32, tag="ot")
                for c in range(8):
                    ops = pspool.tile([128, 512], F32, tag=f"ps{c}",
                                      name=f"out{c}_{blk}")
                    for bloc in range(8):
                        b = 8 * c + bloc
                        h = b // 16
                        k = (b % 16) // 2
                        beta = b % 2
                        oap = ops[:nn, bloc * U:(bloc + 1) * U]
                        mm(oap, xtt[:, b, :nn], k0b_sb[:, :], start=True,
                           stop=False)
                        mm(oap, zts[(0, h)][beta * D:(beta + 1) * D,
                                            128 * k:128 * k + nn],
                           k1_sb[beta * D:(beta + 1) * D, :],
                           start=False, stop=False)
                        mm(oap, zts[(1, h)][beta * D:(beta + 1) * D,
                                            128 * k:128 * k + nn],
                           k2_sb[beta * D:(beta + 1) * D, :],
                           start=False, stop=True)
                    nc.any.tensor_copy(ot[:nn, c * 512:(c + 1) * 512],
                                       ops[:nn, :])

                nc.sync.dma_start(out_t.ap()[n0:n0 + nn, :], ot[:nn, :])
    return nc


def run(inputs, trace=False, **spmd_kwargs):
    supports = [(np.asarray(inputs["sup0_rows"]), np.asarray(inputs["sup0_cols"]),
                 np.asarray(inputs["sup0_vals"], np.float32)),
                (np.asarray(inputs["sup1_rows"]), np.asarray(inputs["sup1_cols"]),
                 np.asarray(inputs["sup1_vals"], np.float32))]
    meta, idx_by_core, sel_by_core = preprocess_edges(supports)
    x0q, xt, k0b, k1, k2, ident = prep_inputs(inputs)

    nc = build_nc(meta)
    nc.compile()
    in_maps = []
    for c in range(NCORES):
        in_maps.append({
            "x0q": x0q,
            "xo": np.ascontiguousarray(xt[:, :, c * NPC:(c + 1) * NPC]),
            "k0b": k0b,
            "k1": k1,
            "k2": k2,
            "ident": ident,
            "idx16": idx_by_core[c],
            "sel": sel_by_core[c],
        })

    from concourse.bass_utils import run_bass_kernel_spmd
    res = run_bass_kernel_spmd(nc, in_maps, core_ids=list(range(NCORES)),
                               trace=trace, **spmd_kwargs)
    out = np.concatenate([np.asarray(res.results[c]["out"])
                          .reshape(NPC, B, U) for c in range(NCORES)], axis=0)
    out = np.ascontiguousarray(out.transpose(1, 0, 2))
    return out, res


def kernel(**inputs) -> np.ndarray:
    out, _ = run(inputs, trace=False)
    return np.asarray(out, np.float32)


# revision 12
# speedup vs baseline: 2.9413x; 1.0343x over previous
"""MGCN Trainium2 kernel v3: direct-X fp8e3 gather, 128-row blocks,
support-split SpMM passes.

Math: out[b] = X[b] @ K0 + bias + A0 @ X[b] @ K1 + A1 @ X[b] @ K2.
The SpMM commutes with the projection, so each core gathers rows of
X0 [N, B*D] quantized to fp8e3m4 on the host (4KB/row). sel matrices stay
bf16 — mixed bf16-stationary x fp8e3-moving matmuls are exact on the PE, so
the only quantization error is e3m4 on X (~1.3% rms; ~1.3e-2 rel measured).

Sharding: node-parallel, core c owns rows [c*1250, (c+1)*1250), processed in
10 blocks of 128 rows. Per block, TWO SpMM passes (support 0 then support 1)
accumulate Z_s [128, 4096] f32 into the same 8 psum banks, 4 row-strips of 32
via tile_position. After each pass the psum drains to SBUF bf16; then 2-batch
PE transposes produce Zt [(parity,d), r] views packed 8-per-bank, and a
per-batch projection (3 matmuls: X-part K=65, Z1 K=64, Z2 K=64) writes
out chunks, stored [NPC, B*U] so each block's result is one contiguous DMA.
"""

import numpy as np
import ml_dtypes

import concourse.bass as bass
import concourse.bacc as bacc
import concourse.mybir as mybir
from concourse.tile import TileContext, add_dep_helper

F32 = mybir.dt.float32
BF16 = mybir.dt.bfloat16
FP8E3 = mybir.dt.float8e3
I16 = mybir.dt.int16

B, N, D, U = 64, 10000, 64, 64
NCORES = 8
NPC = N // NCORES            # 1250 rows per core
BLK = 128                    # block rows
NB = (NPC + BLK - 1) // BLK  # 10 blocks (last has 98 rows)
F = B * D                    # 4096 gather-row features
KD = D + 1                   # contraction incl. ones row
NCHUNK = F // 512            # 8 psum column chunks
GU = 4                       # gather unit: tiles of 128 edges per dma_gather
NQ = 4


class Meta:
    pass


def preprocess_edges(supports):
    """Bucket edges by (core, blk, s, j) with j = (row%128)//32, pad to a
    uniform per-(blk,s,j) tile count across cores (SPMD)."""
    groups = {}
    for s, (rows, cols, vals) in enumerate(supports):
        rows = np.asarray(rows)
        cols = np.asarray(cols)
        vals = np.asarray(vals, np.float32)
        order = np.argsort(rows, kind="stable")
        r, c, v = rows[order], cols[order], vals[order]
        core = r // NPC
        rr = r % NPC
        blk = rr // BLK
        j = (rr % BLK) // 32
        lr = rr % 32
        for cc in range(NCORES):
            m0 = core == cc
            for bb in range(NB):
                m1 = m0 & (blk == bb)
                for jj in range(4):
                    m = m1 & (j == jj)
                    if not m.any():
                        continue
                    g = groups.setdefault((cc, bb, s, jj), [[], [], []])
                    g[0].append(c[m])
                    g[1].append(v[m])
                    g[2].append(lr[m])

    def glen(key):
        g = groups.get(key)
        return sum(len(a) for a in g[0]) if g else 0

    # T[blk][s][j]
    T = [[[0] * 4 for _ in range(2)] for _ in range(NB)]
    for bb in range(NB):
        for s in range(2):
            for jj in range(4):
                mx = max(glen((cc, bb, s, jj)) for cc in range(NCORES))
                T[bb][s][jj] = (mx + 127) // 128

    idx_off = [[[0] * 4 for _ in range(2)] for _ in range(NB)]
    sel_off = [[[0] * 4 for _ in range(2)] for _ in range(NB)]
    io = so = 0
    for bb in range(NB):
        for s in range(2):
            for jj in range(4):
                idx_off[bb][s][jj] = io
                sel_off[bb][s][jj] = so
                io += T[bb][s][jj] * 8
                so += T[bb][s][jj] * 32

    idx_by_core, sel_by_core = [], []
    for cc in range(NCORES):
        idx_cols, sel_cols = [], []
        for bb in range(NB):
            for s in range(2):
                for jj in range(4):
                    Tt = T[bb][s][jj]
                    if Tt == 0:
                        continue
                    g = groups.get((cc, bb, s, jj))
                    if g is None:
                        gi = np.zeros(0, np.int64)
                        gv = np.zeros(0, np.float32)
                        gl = np.zeros(0, np.int64)
                    else:
                        gi = np.concatenate(g[0])
                        gv = np.concatenate(g[1])
                        gl = np.concatenate(g[2])
                    pad = Tt * 128 - len(gi)
                    gi = np.concatenate([gi, np.zeros(pad, np.int64)])
                    gv = np.concatenate([gv, np.zeros(pad, np.float32)])
                    gl = np.concatenate([gl, np.zeros(pad, np.int64)])
                    # idx wrap: index i -> [i % 16, i // 16], replicated x8
                    wrapped = gi.astype(np.int16).reshape(Tt * 8, 16).T
                    idx_cols.append(np.tile(wrapped, (8, 1)))
                    sel = np.zeros((128, Tt, 32), np.float32)
                    lane = np.arange(Tt * 128) % 128
                    tt = np.arange(Tt * 128) // 128
                    sel[lane, tt, gl] = gv
                    sel_cols.append(sel.reshape(128, Tt * 32)
                                    .astype(ml_dtypes.bfloat16))
        idx_by_core.append(np.ascontiguousarray(
            np.concatenate(idx_cols, axis=1)))
        sel_by_core.append(np.ascontiguousarray(
            np.concatenate(sel_cols, axis=1)))

    meta = Meta()
    meta.T = T
    meta.idx_off = idx_off
    meta.sel_off = sel_off
    meta.idx_shape = idx_by_core[0].shape
    meta.sel_shape = sel_by_core[0].shape
    meta.blk_idx_off = [idx_off[bb][0][0] for bb in range(NB)]
    meta.blk_idx_w = [sum(T[bb][s][jj] * 8 for s in range(2)
                          for jj in range(4)) for bb in range(NB)]
    meta.TQmax = max(T[bb][s][jj] for bb in range(NB) for s in range(2)
                     for jj in range(4))
    meta.IWmax = max(meta.blk_idx_w)
    return meta, idx_by_core, sel_by_core


def prep_inputs(inputs):
    x = np.asarray(inputs["x"], np.float32)
    kernel = np.asarray(inputs["kernel"], np.float32)
    bias = np.asarray(inputs["bias"], np.float32)

    x0 = np.ascontiguousarray(x.transpose(1, 0, 2).reshape(N, B * D))
    x0q = x0.astype(ml_dtypes.float8_e3m4)

    xt = np.empty((KD, B, N), np.float32)
    xt[:D] = x.transpose(2, 0, 1)
    xt[D] = 1.0
    xt = xt.astype(ml_dtypes.bfloat16)

    K = kernel.reshape(D, 3, U)
    k0b = np.zeros((KD, U), np.float32)
    k0b[:D] = K[:, 0]
    k0b[D] = bias
    # duplicated along partitions so rhs base_partition can match the
    # lhsT parity offset (0 or 64) in the projection matmuls
    k1 = np.ascontiguousarray(np.vstack([K[:, 1], K[:, 1]]))
    k2 = np.ascontiguousarray(np.vstack([K[:, 2], K[:, 2]]))
    ident = np.eye(128, dtype=np.float32)
    return (x0q, xt,
            k0b.astype(ml_dtypes.bfloat16), k1.astype(ml_dtypes.bfloat16),
            k2.astype(ml_dtypes.bfloat16), ident.astype(ml_dtypes.bfloat16))


def build_nc(meta):
    nc = bacc.Bacc("TRN2", num_devices=NCORES,
                   dynamic_dma_scratch_size=16384,
                   num_swdge_queues=NQ)

    x0q_t = nc.dram_tensor("x0q", [N, F], FP8E3, kind="ExternalInput")
    xo_t = nc.dram_tensor("xo", [KD, B, NPC], BF16, kind="ExternalInput")
    k0b_t = nc.dram_tensor("k0b", [KD, U], BF16, kind="ExternalInput")
    k1_t = nc.dram_tensor("k1", [2 * D, U], BF16, kind="ExternalInput")
    k2_t = nc.dram_tensor("k2", [2 * D, U], BF16, kind="ExternalInput")
    id_t = nc.dram_tensor("ident", [128, 128], BF16, kind="ExternalInput")
    idx_t = nc.dram_tensor("idx16", list(meta.idx_shape), I16,
                           kind="ExternalInput")
    sel_t = nc.dram_tensor("sel", list(meta.sel_shape), BF16,
                           kind="ExternalInput")
    out_t = nc.dram_tensor("out", [NPC, B * U], BF16, kind="ExternalOutput")

    with TileContext(nc) as tc:
        with tc.tile_pool(name="kpool", bufs=1) as kpool, \
             tc.tile_pool(name="gp", bufs=7) as gpool, \
             tc.tile_pool(name="ip", bufs=2) as ipool, \
             tc.tile_pool(name="sp", bufs=10) as spool, \
             tc.tile_pool(name="xb", bufs=2) as xbpool, \
             tc.tile_pool(name="zb", bufs=1) as zbpool, \
             tc.tile_pool(name="zt", bufs=8) as ztpool, \
             tc.tile_pool(name="op", bufs=2) as opool, \
             tc.tile_pool(name="ps", bufs=1, space="PSUM") as pspool:

            k0b_sb = kpool.tile([KD, U], BF16, tag="k0b")
            nc.sync.dma_start(k0b_sb[:, :], k0b_t.ap()[:, :])
            k1_sb = kpool.tile([2 * D, U], BF16, tag="k1")
            nc.sync.dma_start(k1_sb[:, :], k1_t.ap()[:, :])
            k2_sb = kpool.tile([2 * D, U], BF16, tag="k2")
            nc.sync.dma_start(k2_sb[:, :], k2_t.ap()[:, :])
            id_sb = kpool.tile([128, 128], BF16, tag="id")
            nc.sync.dma_start(id_sb[:, :], id_t.ap()[:, :])

            prev_mm = [None]

            def mm(*args, **kwargs):
                m = nc.tensor.matmul(*args, skip_group_check=True, **kwargs)
                if prev_mm[0] is not None:
                    add_dep_helper(m.ins, prev_mm[0].ins, sync=False,
                                   reason="pe order")
                prev_mm[0] = m
                return m

            gq = 0
            for blk in range(NB):
                n0 = blk * BLK
                nn = min(BLK, NPC - n0)

                xtt = xbpool.tile([KD, B, BLK], BF16, tag="xtt")
                nc.sync.dma_start(xtt[:, :, :nn],
                                  xo_t.ap()[:, :, n0:n0 + nn])

                iw = meta.blk_idx_w[blk]
                it = ipool.tile([128, meta.IWmax], I16, tag="idx")
                nc.sync.dma_start(it[:, :iw],
                                  idx_t.ap()[:, meta.blk_idx_off[blk]:
                                             meta.blk_idx_off[blk] + iw])

                zsbs = {}
                for s in range(2):
                    Ths = meta.T[blk][s]
                    qs = [q for q in range(4) if Ths[q] > 0]
                    pss = [pspool.tile([128, 512], F32, tag=f"ps{f}",
                                       name=f"z{s}c{f}_{blk}")
                           for f in range(NCHUNK)]

                    sls = {}
                    for q in qs:
                        sl = spool.tile([128, meta.TQmax * 32], BF16,
                                        tag="sel")
                        so = meta.sel_off[blk][s][q]
                        nc.sync.dma_start(sl[:, :Ths[q] * 32],
                                          sel_t.ap()[:, so:so + Ths[q] * 32])
                        sls[q] = sl

                    mm_specs = {q: [] for q in qs}
                    units = {q: list(range(0, Ths[q], GU)) for q in qs}
                    for k in range(max(len(u) for u in units.values())):
                        for q in qs:
                            if k >= len(units[q]):
                                continue
                            u0 = units[q][k]
                            nt = min(GU, Ths[q] - u0)
                            io = (meta.idx_off[blk][s][q]
                                  - meta.blk_idx_off[blk] + u0 * 8)
                            gt = gpool.tile([128, GU, F], FP8E3, tag="g")
                            nc.gpsimd.dma_gather(
                                gt[:, :nt, :], x0q_t.ap()[:, :],
                                it[:, io:io + nt * 8],
                                num_idxs=nt * 128, num_idxs_reg=nt * 128,
                                elem_size=F, queue_num=gq % NQ)
                            gq += 1
                            for ti in range(nt):
                                for f in range(NCHUNK):
                                    mm_specs[q].append(
                                        (sls[q][:, (u0 + ti) * 32:
                                                (u0 + ti + 1) * 32],
                                         gt[:, ti, f * 512:(f + 1) * 512], f))

                    idxs = {q: 0 for q in qs}
                    cnt = {}
                    total = {q: len(mm_specs[q]) for q in qs}
                    remaining = sum(total.values())
                    while remaining:
                        for q in qs:
                            i = idxs[q]
                            if i >= total[q]:
                                continue
                            sel_ap, g_ap, f = mm_specs[q][i]
                            c = cnt.get((q, f), 0)
                            nmm = total[q] // NCHUNK
                            mm(pss[f][32 * q:32 * (q + 1), :], sel_ap, g_ap,
                               start=(c == 0), stop=(c == nmm - 1),
                               tile_position=(0, 32 * q))
                            cnt[(q, f)] = c + 1
                            idxs[q] += 1
                            remaining -= 1

                    zsb = zbpool.tile([128, F], BF16, tag=f"zsb{s}")
                    for f in range(NCHUNK):
                        nc.any.tensor_copy(zsb[:, f * 512:(f + 1) * 512],
                                           pss[f][:, :])
                    zsbs[s] = zsb

                # 2-batch transposes: zt psum view [128, 1024] bf16 packs 8
                # transposes = 16 batches; tags: s=0 -> ps0..3, s=1 -> ps4..7
                zts = {}
                for s in range(2):
                    for h in range(4):
                        ztp = pspool.tile([128, 512], F32,
                                          tag=f"ps{4 * s + h}",
                                          name=f"zt{s}_{h}_{blk}")
                        ztv = ztp[:, :].bitcast(BF16)
                        for k in range(8):
                            b2 = 16 * h + 2 * k
                            mm(ztv[:, 128 * k:128 * (k + 1)],
                               zsbs[s][:, b2 * D:(b2 + 2) * D], id_sb[:, :],
                               is_transpose=True)
                        zs = ztpool.tile([128, 1024], BF16, tag="zt")
                        nc.any.tensor_copy(zs[:, :], ztv[:, :])
                        zts[(s, h)] = zs

                # projection: out chunk c serves batches 8c..8c+7
                ot = opool.tile([BLK, F], BF16, tag="ot")
                for c in range(8):
                    ops = pspool.tile([128, 512], F32, tag=f"ps{c}",
                                      name=f"out{c}_{blk}")
                    for bloc in range(8):
                        b = 8 * c + bloc
                        h = b // 16
                        k = (b % 16) // 2
                        beta = b % 2
                        oap = ops[:nn, bloc * U:(bloc + 1) * U]
                        mm(oap, xtt[:, b, :nn], k0b_sb[:, :], start=True,
                           stop=False)
                        mm(oap, zts[(0, h)][beta * D:(beta + 1) * D,
                                            128 * k:128 * k + nn],
                           k1_sb[beta * D:(beta + 1) * D, :],
                           start=False, stop=False)
                        mm(oap, zts[(1, h)][beta * D:(beta + 1) * D,
                                            128 * k:128 * k + nn],
                           k2_sb[beta * D:(beta + 1) * D, :],
                           start=False, stop=True)
                    nc.any.tensor_copy(ot[:nn, c * 512:(c + 1) * 512],
                                       ops[:nn, :])

                nc.sync.dma_start(out_t.ap()[n0:n0 + nn, :], ot[:nn, :])
    return nc


def run(inputs, trace=False, **spmd_kwargs):
    supports = [(np.asarray(inputs["sup0_rows"]), np.asarray(inputs["sup0_cols"]),
                 np.asarray(inputs["sup0_vals"], np.float32)),
                (np.asarray(inputs["sup1_rows"]), np.asarray(inputs["sup1_cols"]),
                 np.asarray(inputs["sup1_vals"], np.float32))]
    meta, idx_by_core, sel_by_core = preprocess_edges(supports)
    x0q, xt, k0b, k1, k2, ident = prep_inputs(inputs)

    nc = build_nc(meta)
    nc.compile()
    in_maps = []
    for c in range(NCORES):
        in_maps.append({
            "x0q": x0q,
            "xo": np.ascontiguousarray(xt[:, :, c * NPC:(c + 1) * NPC]),
            "k0b": k0b,
            "k1": k1,
            "k2": k2,
            "ident": ident,
            "idx16": idx_by_core[c],
            "sel": sel_by_core[c],
        })

    from concourse.bass_utils import run_bass_kernel_spmd
    res = run_bass_kernel_spmd(nc, in_maps, core_ids=list(range(NCORES)),
                               trace=trace, **spmd_kwargs)
    out = np.concatenate([np.asarray(res.results[c]["out"])
                          .astype(np.float32)
                          .reshape(NPC, B, U) for c in range(NCORES)], axis=0)
    out = np.ascontiguousarray(out.transpose(1, 0, 2))
    return out, res


def kernel(**inputs) -> np.ndarray:
    out, _ = run(inputs, trace=False)
    return np.asarray(out, np.float32)


# revision 13
# speedup vs baseline: 3.3558x; 1.1409x over previous
"""MGCN Trainium2 kernel v3: direct-X fp8e3 gather, 128-row blocks,
support-split SpMM passes.

Math: out[b] = X[b] @ K0 + bias + A0 @ X[b] @ K1 + A1 @ X[b] @ K2.
The SpMM commutes with the projection, so each core gathers rows of
X0 [N, B*D] quantized to fp8e3m4 on the host (4KB/row). sel matrices stay
bf16 — mixed bf16-stationary x fp8e3-moving matmuls are exact on the PE, so
the only quantization error is e3m4 on X (~1.3% rms; ~1.3e-2 rel measured).

Sharding: node-parallel, core c owns rows [c*1250, (c+1)*1250), processed in
10 blocks of 128 rows. Per block, TWO SpMM passes (support 0 then support 1)
accumulate Z_s [128, 4096] f32 into the same 8 psum banks, 4 row-strips of 32
via tile_position. After each pass the psum drains to SBUF bf16; then 2-batch
PE transposes produce Zt [(parity,d), r] views packed 8-per-bank, and a
per-batch projection (3 matmuls: X-part K=65, Z1 K=64, Z2 K=64) writes
out chunks, stored [NPC, B*U] so each block's result is one contiguous DMA.
"""

import numpy as np
import ml_dtypes

import concourse.bass as bass
import concourse.bacc as bacc
import concourse.mybir as mybir
from concourse.tile import TileContext, add_dep_helper

F32 = mybir.dt.float32
BF16 = mybir.dt.bfloat16
FP8E3 = mybir.dt.float8e3
I16 = mybir.dt.int16

B, N, D, U = 64, 10000, 64, 64
NCORES = 8
NPC = N // NCORES            # 1250 rows per core
BLK = 128                    # block rows
NB = (NPC + BLK - 1) // BLK  # 10 blocks (last has 98 rows)
F = B * D                    # 4096 gather-row features
KD = D + 1                   # contraction incl. ones row
NCHUNK = F // 512            # 8 psum column chunks
GU = 4                       # gather unit: tiles of 128 edges per dma_gather
NQ = 4


class Meta:
    pass


def preprocess_edges(supports):
    """Bucket edges by (core, blk, s, j) with j = (row%128)//32, pad to a
    uniform per-(blk,s,j) tile count across cores (SPMD)."""
    groups = {}
    for s, (rows, cols, vals) in enumerate(supports):
        rows = np.asarray(rows)
        cols = np.asarray(cols)
        vals = np.asarray(vals, np.float32)
        order = np.argsort(rows, kind="stable")
        r, c, v = rows[order], cols[order], vals[order]
        core = r // NPC
        rr = r % NPC
        blk = rr // BLK
        j = (rr % BLK) // 32
        lr = rr % 32
        for cc in range(NCORES):
            m0 = core == cc
            for bb in range(NB):
                m1 = m0 & (blk == bb)
                for jj in range(4):
                    m = m1 & (j == jj)
                    if not m.any():
                        continue
                    g = groups.setdefault((cc, bb, s, jj), [[], [], []])
                    g[0].append(c[m])
                    g[1].append(v[m])
                    g[2].append(lr[m])

    def glen(key):
        g = groups.get(key)
        return sum(len(a) for a in g[0]) if g else 0

    # T[blk][s][j]
    T = [[[0] * 4 for _ in range(2)] for _ in range(NB)]
    for bb in range(NB):
        for s in range(2):
            for jj in range(4):
                mx = max(glen((cc, bb, s, jj)) for cc in range(NCORES))
                T[bb][s][jj] = (mx + 127) // 128

    idx_off = [[[0] * 4 for _ in range(2)] for _ in range(NB)]
    sel_off = [[[0] * 4 for _ in range(2)] for _ in range(NB)]
    io = so = 0
    for bb in range(NB):
        for s in range(2):
            for jj in range(4):
                idx_off[bb][s][jj] = io
                sel_off[bb][s][jj] = so
                io += T[bb][s][jj] * 8
                so += T[bb][s][jj] * 32

    idx_by_core, sel_by_core = [], []
    for cc in range(NCORES):
        idx_cols, sel_cols = [], []
        for bb in range(NB):
            for s in range(2):
                for jj in range(4):
                    Tt = T[bb][s][jj]
                    if Tt == 0:
                        continue
                    g = groups.get((cc, bb, s, jj))
                    if g is None:
                        gi = np.zeros(0, np.int64)
                        gv = np.zeros(0, np.float32)
                        gl = np.zeros(0, np.int64)
                    else:
                        gi = np.concatenate(g[0])
                        gv = np.concatenate(g[1])
                        gl = np.concatenate(g[2])
                    pad = Tt * 128 - len(gi)
                    gi = np.concatenate([gi, np.zeros(pad, np.int64)])
                    gv = np.concatenate([gv, np.zeros(pad, np.float32)])
                    gl = np.concatenate([gl, np.zeros(pad, np.int64)])
                    # idx wrap: index i -> [i % 16, i // 16], replicated x8
                    wrapped = gi.astype(np.int16).reshape(Tt * 8, 16).T
                    idx_cols.append(np.tile(wrapped, (8, 1)))
                    sel = np.zeros((128, Tt, 32), np.float32)
                    lane = np.arange(Tt * 128) % 128
                    tt = np.arange(Tt * 128) // 128
                    sel[lane, tt, gl] = gv
                    sel_cols.append(sel.reshape(128, Tt * 32)
                                    .astype(ml_dtypes.bfloat16))
        idx_by_core.append(np.ascontiguousarray(
            np.concatenate(idx_cols, axis=1)))
        sel_by_core.append(np.ascontiguousarray(
            np.concatenate(sel_cols, axis=1)))

    meta = Meta()
    meta.T = T
    meta.idx_off = idx_off
    meta.sel_off = sel_off
    meta.idx_shape = idx_by_core[0].shape
    meta.sel_shape = sel_by_core[0].shape
    meta.blk_idx_off = [idx_off[bb][0][0] for bb in range(NB)]
    meta.blk_idx_w = [sum(T[bb][s][jj] * 8 for s in range(2)
                          for jj in range(4)) for bb in range(NB)]
    meta.TQmax = max(T[bb][s][jj] for bb in range(NB) for s in range(2)
                     for jj in range(4))
    meta.IWmax = max(meta.blk_idx_w)
    return meta, idx_by_core, sel_by_core


def prep_inputs(inputs):
    x = np.asarray(inputs["x"], np.float32)
    kernel = np.asarray(inputs["kernel"], np.float32)
    bias = np.asarray(inputs["bias"], np.float32)

    x0 = np.ascontiguousarray(x.transpose(1, 0, 2).reshape(N, B * D))
    x0q = x0.astype(ml_dtypes.float8_e3m4)

    xt = np.empty((KD, B, N), np.float32)
    xt[:D] = x.transpose(2, 0, 1)
    xt[D] = 1.0
    xt = xt.astype(ml_dtypes.bfloat16)

    K = kernel.reshape(D, 3, U)
    k0b = np.zeros((KD, U), np.float32)
    k0b[:D] = K[:, 0]
    k0b[D] = bias
    # duplicated along partitions so rhs base_partition can match the
    # lhsT parity offset (0 or 64) in the projection matmuls
    k1 = np.ascontiguousarray(np.vstack([K[:, 1], K[:, 1]]))
    k2 = np.ascontiguousarray(np.vstack([K[:, 2], K[:, 2]]))
    ident = np.eye(128, dtype=np.float32)
    return (x0q, xt,
            k0b.astype(ml_dtypes.bfloat16), k1.astype(ml_dtypes.bfloat16),
            k2.astype(ml_dtypes.bfloat16), ident.astype(ml_dtypes.bfloat16))


def build_nc(meta):
    nc = bacc.Bacc("TRN2", num_devices=NCORES,
                   dynamic_dma_scratch_size=16384,
                   num_swdge_queues=NQ)

    x0q_t = nc.dram_tensor("x0q", [N, F], FP8E3, kind="ExternalInput")
    xo_t = nc.dram_tensor("xo", [KD, B, NPC], BF16, kind="ExternalInput")
    k0b_t = nc.dram_tensor("k0b", [KD, U], BF16, kind="ExternalInput")
    k1_t = nc.dram_tensor("k1", [2 * D, U], BF16, kind="ExternalInput")
    k2_t = nc.dram_tensor("k2", [2 * D, U], BF16, kind="ExternalInput")
    id_t = nc.dram_tensor("ident", [128, 128], BF16, kind="ExternalInput")
    idx_t = nc.dram_tensor("idx16", list(meta.idx_shape), I16,
                           kind="ExternalInput")
    sel_t = nc.dram_tensor("sel", list(meta.sel_shape), BF16,
                           kind="ExternalInput")
    out_t = nc.dram_tensor("out", [NPC, B * U], BF16, kind="ExternalOutput")

    with TileContext(nc) as tc:
        with tc.tile_pool(name="kpool", bufs=1) as kpool, \
             tc.tile_pool(name="gp", bufs=8) as gpool, \
             tc.tile_pool(name="ip", bufs=2) as ipool, \
             tc.tile_pool(name="sp", bufs=10) as spool, \
             tc.tile_pool(name="xb", bufs=1) as xbpool, \
             tc.tile_pool(name="zb", bufs=1) as zbpool, \
             tc.tile_pool(name="zt", bufs=8) as ztpool, \
             tc.tile_pool(name="op", bufs=2) as opool, \
             tc.tile_pool(name="ps", bufs=1, space="PSUM") as pspool:

            k0b_sb = kpool.tile([KD, U], BF16, tag="k0b")
            nc.sync.dma_start(k0b_sb[:, :], k0b_t.ap()[:, :])
            k1_sb = kpool.tile([2 * D, U], BF16, tag="k1")
            nc.sync.dma_start(k1_sb[:, :], k1_t.ap()[:, :])
            k2_sb = kpool.tile([2 * D, U], BF16, tag="k2")
            nc.sync.dma_start(k2_sb[:, :], k2_t.ap()[:, :])
            id_sb = kpool.tile([128, 128], BF16, tag="id")
            nc.sync.dma_start(id_sb[:, :], id_t.ap()[:, :])

            prev_mm = [None]

            def mm(*args, **kwargs):
                m = nc.tensor.matmul(*args, skip_group_check=True, **kwargs)
                if prev_mm[0] is not None:
                    add_dep_helper(m.ins, prev_mm[0].ins, sync=False,
                                   reason="pe order")
                prev_mm[0] = m
                return m

            gq = 0
            for blk in range(NB):
                n0 = blk * BLK
                nn = min(BLK, NPC - n0)

                xtt = xbpool.tile([KD, B, BLK], BF16, tag="xtt")
                nc.sync.dma_start(xtt[:, :, :nn],
                                  xo_t.ap()[:, :, n0:n0 + nn])

                iw = meta.blk_idx_w[blk]
                it = ipool.tile([128, meta.IWmax], I16, tag="idx")
                nc.sync.dma_start(it[:, :iw],
                                  idx_t.ap()[:, meta.blk_idx_off[blk]:
                                             meta.blk_idx_off[blk] + iw])

                zsbs = {}
                for s in range(2):
                    Ths = meta.T[blk][s]
                    qs = [q for q in range(4) if Ths[q] > 0]
                    pss = [pspool.tile([128, 512], F32, tag=f"ps{f}",
                                       name=f"z{s}c{f}_{blk}")
                           for f in range(NCHUNK)]

                    sls = {}
                    for q in qs:
                        sl = spool.tile([128, meta.TQmax * 32], BF16,
                                        tag="sel")
                        so = meta.sel_off[blk][s][q]
                        nc.sync.dma_start(sl[:, :Ths[q] * 32],
                                          sel_t.ap()[:, so:so + Ths[q] * 32])
                        sls[q] = sl

                    mm_specs = {q: [] for q in qs}
                    units = {q: list(range(0, Ths[q], GU)) for q in qs}
                    for k in range(max(len(u) for u in units.values())):
                        for q in qs:
                            if k >= len(units[q]):
                                continue
                            u0 = units[q][k]
                            nt = min(GU, Ths[q] - u0)
                            io = (meta.idx_off[blk][s][q]
                                  - meta.blk_idx_off[blk] + u0 * 8)
                            gt = gpool.tile([128, GU, F], FP8E3, tag="g")
                            nc.gpsimd.dma_gather(
                                gt[:, :nt, :], x0q_t.ap()[:, :],
                                it[:, io:io + nt * 8],
                                num_idxs=nt * 128, num_idxs_reg=nt * 128,
                                elem_size=F, queue_num=gq % NQ)
                            gq += 1
                            for ti in range(nt):
                                for f in range(NCHUNK):
                                    mm_specs[q].append(
                                        (sls[q][:, (u0 + ti) * 32:
                                                (u0 + ti + 1) * 32],
                                         gt[:, ti, f * 512:(f + 1) * 512], f))

                    idxs = {q: 0 for q in qs}
                    cnt = {}
                    total = {q: len(mm_specs[q]) for q in qs}
                    remaining = sum(total.values())
                    while remaining:
                        for q in qs:
                            i = idxs[q]
                            if i >= total[q]:
                                continue
                            sel_ap, g_ap, f = mm_specs[q][i]
                            c = cnt.get((q, f), 0)
                            nmm = total[q] // NCHUNK
                            mm(pss[f][32 * q:32 * (q + 1), :], sel_ap, g_ap,
                               start=(c == 0), stop=(c == nmm - 1),
                               tile_position=(0, 32 * q))
                            cnt[(q, f)] = c + 1
                            idxs[q] += 1
                            remaining -= 1

                    zsb = zbpool.tile([128, F], BF16, tag=f"zsb{s}")
                    for f in range(NCHUNK):
                        nc.any.tensor_copy(zsb[:, f * 512:(f + 1) * 512],
                                           pss[f][:, :])
                    zsbs[s] = zsb

                # 2-batch transposes: zt psum view [128, 1024] bf16 packs 8
                # transposes = 16 batches; tags: s=0 -> ps0..3, s=1 -> ps4..7
                zts = {}
                for s in range(2):
                    for h in range(4):
                        ztp = pspool.tile([128, 512], F32,
                                          tag=f"ps{4 * s + h}",
                                          name=f"zt{s}_{h}_{blk}")
                        ztv = ztp[:, :].bitcast(BF16)
                        for k in range(8):
                            b2 = 16 * h + 2 * k
                            mm(ztv[:, 128 * k:128 * (k + 1)],
                               zsbs[s][:, b2 * D:(b2 + 2) * D], id_sb[:, :],
                               is_transpose=True)
                        zs = ztpool.tile([128, 1024], BF16, tag="zt")
                        nc.any.tensor_copy(zs[:, :], ztv[:, :])
                        zts[(s, h)] = zs

                # projection: out chunk c serves batches 8c..8c+7
                ot = opool.tile([BLK, F], BF16, tag="ot")
                for c in range(8):
                    ops = pspool.tile([128, 512], F32, tag=f"ps{c}",
                                      name=f"out{c}_{blk}")
                    for bloc in range(8):
                        b = 8 * c + bloc
                        h = b // 16
                        k = (b % 16) // 2
                        beta = b % 2
                        oap = ops[:nn, bloc * U:(bloc + 1) * U]
                        mm(oap, xtt[:, b, :nn], k0b_sb[:, :], start=True,
                           stop=False)
                        mm(oap, zts[(0, h)][beta * D:(beta + 1) * D,
                                            128 * k:128 * k + nn],
                           k1_sb[beta * D:(beta + 1) * D, :],
                           start=False, stop=False)
                        mm(oap, zts[(1, h)][beta * D:(beta + 1) * D,
                                            128 * k:128 * k + nn],
                           k2_sb[beta * D:(beta + 1) * D, :],
                           start=False, stop=True)
                    nc.any.tensor_copy(ot[:nn, c * 512:(c + 1) * 512],
                                       ops[:nn, :])

                nc.sync.dma_start(out_t.ap()[n0:n0 + nn, :], ot[:nn, :])
    return nc


def run(inputs, trace=False, **spmd_kwargs):
    supports = [(np.asarray(inputs["sup0_rows"]), np.asarray(inputs["sup0_cols"]),
                 np.asarray(inputs["sup0_vals"], np.float32)),
                (np.asarray(inputs["sup1_rows"]), np.asarray(inputs["sup1_cols"]),
                 np.asarray(inputs["sup1_vals"], np.float32))]
    meta, idx_by_core, sel_by_core = preprocess_edges(supports)
    x0q, xt, k0b, k1, k2, ident = prep_inputs(inputs)

    nc = build_nc(meta)
    nc.compile()
    in_maps = []
    for c in range(NCORES):
        in_maps.append({
            "x0q": x0q,
            "xo": np.ascontiguousarray(xt[:, :, c * NPC:(c + 1) * NPC]),
            "k0b": k0b,
            "k1": k1,
            "k2": k2,
            "ident": ident,
            "idx16": idx_by_core[c],
            "sel": sel_by_core[c],
        })

    from concourse.bass_utils import run_bass_kernel_spmd
    res = run_bass_kernel_spmd(nc, in_maps, core_ids=list(range(NCORES)),
                               trace=trace, **spmd_kwargs)
    out = np.concatenate([np.asarray(res.results[c]["out"])
                          .astype(np.float32)
                          .reshape(NPC, B, U) for c in range(NCORES)], axis=0)
    out = np.ascontiguousarray(out.transpose(1, 0, 2))
    return out, res


def kernel(**inputs) -> np.ndarray:
    out, _ = run(inputs, trace=False)
    return np.asarray(out, np.float32)
